# revision 24
# baseline (speedup 1.0000x reference)
"""Causal self-attention (dense transformer block) on 8 Trainium2 NeuronCores.

Problem (hardcoded): B=2, S=2048, HID=2048, NH=16, HS=128, ROT=32 (partial
rotary), causal additive mask, f32 I/O.

Sharding: core c = b*4 + g handles batch b and head-group g (4 heads).
 - Phase A (projections): qkT = (x @ Wqk_shard)^T computed from host-provided
   xT (x[b] transposed, pre-cast bf16) so the contraction dim lands on
   partitions.  Q,K are produced transposed ([d, s]); V natural ([s, d]).
 - RoPE in-place on the first 32 dims of each head of Q,K; rotate_half is a
   32x32 sign-permutation matmul (engine partition ops must be 32-aligned).
 - Phase B (attention): S^T tiles [128 k, 512 q] on PE, exp on ACT (no max
   subtraction; scores are O(5)), causality structural (upper blocks skipped,
   diagonal blocks masked after exp), AV and row-sum accumulated on PE
   (row-sum via all-ones 128x128 stationary => result replicated across
   partitions, no broadcast needed).
 - Per-q-block AllGather (groups of 4 = same batch) overlapped with later
   compute phases; dense (column-parallel Wdense) per q-block right after.
 - Host reassembles the 8 [512, 2048] f32 transposed output slices.
"""

import numpy as np
import ml_dtypes

import concourse.bass as bass
import concourse.bacc as bacc
import concourse.mybir as mybir
import concourse.tile as tile
from concourse import bass_utils

B, S, HID = 2, 2048, 2048
NH = 16
HS = 128
ROT = 32
BASE = 10000.0
G = 4            # head-groups (4 heads each)
HPG = NH // G    # heads per core = 4
NCORES = 8

MB = 4           # m (seq) blocks of 512
KB = 16          # contraction blocks of 128
NBQK = 2 * HPG   # qk feature blocks of 128 (q,k interleaved per head)
F32 = mybir.dt.float32
BF16 = mybir.dt.bfloat16
SCALE = 1.0 / float(np.sqrt(np.float32(HS)))

_CACHE = {}


def _build_nc():
    nc = bacc.Bacc(
        "TRN2", target_bir_lowering=False, debug=False, num_devices=NCORES
    )

    io = {
        "xT": nc.dram_tensor("xT", [HID, S], BF16, kind="ExternalInput"),
        # wqk host-layout: [nb, p, kb*128+n] so each nb is one contiguous DMA
        "wqk": nc.dram_tensor("wqk", [NBQK, 128, KB * 128], BF16, kind="ExternalInput"),
        "wv": nc.dram_tensor("wv", [KB, 128, 512], BF16, kind="ExternalInput"),
        "wd": nc.dram_tensor("wd", [KB, 128, 512], BF16, kind="ExternalInput"),
        "bqk": nc.dram_tensor("bqk", [NBQK * 128], F32, kind="ExternalInput"),
        "bv": nc.dram_tensor("bv", [HPG * 128], F32, kind="ExternalInput"),
        "bd": nc.dram_tensor("bd", [512], F32, kind="ExternalInput"),
        "cosb": nc.dram_tensor("cosb", [ROT, S], BF16, kind="ExternalInput"),
        "sinb": nc.dram_tensor("sinb", [ROT, S], BF16, kind="ExternalInput"),
        "rt": nc.dram_tensor("rt", [ROT, ROT], BF16, kind="ExternalInput"),
        "masks": nc.dram_tensor("masks", [128, 4, 512], BF16, kind="ExternalInput"),
        "outT": nc.dram_tensor("outT", [512, S], F32, kind="ExternalOutput"),
    }

    with tile.TileContext(nc) as tc:
        _kernel_body(tc, io)

    nc.compile()
    return nc


def _kernel_body(tc, io):
    nc = tc.nc
    from contextlib import ExitStack

    with ExitStack() as ctx:
        singles = ctx.enter_context(tc.tile_pool(name="singles", bufs=1))
        wpool = ctx.enter_context(tc.tile_pool(name="wpool", bufs=1))
        work = ctx.enter_context(tc.tile_pool(name="work", bufs=2))
        psum = ctx.enter_context(tc.tile_pool(name="psum", bufs=2, space="PSUM"))
        dram = ctx.enter_context(tc.tile_pool(name="dram", bufs=1, space="DRAM"))

        # ---- weights for the very first matmuls, in load order ----
        # (first qk weight block, then the first m-block of x, then the rest:
        # the first PE matmul needs only wqk[0] + xbf(0))
        wqk_sb = []
        for nb in range(NBQK):
            w_t = wpool.tile([128, KB, 128], BF16, tag=f"wqk{nb}", name=f"wqk{nb}")
            nc.sync.dma_start(out=w_t, in_=io["wqk"][nb])
            wqk_sb.append(w_t)
            if nb == 0:
                xbf0 = []
                for kb in range(KB):
                    xb = work.tile([128, 512], BF16, tag="xbf", bufs=18, name="xb")
                    nc.sync.dma_start(
                        out=xb, in_=io["xT"][128 * kb : 128 * (kb + 1), 0:512]
                    )
                    xbf0.append(xb)

        # ---- constants ----
        cos_sb = singles.tile([ROT, S], BF16, tag="cos_sb")
        nc.sync.dma_start(out=cos_sb, in_=io["cosb"][:, :])
        sin_sb = singles.tile([ROT, S], BF16, tag="sin_sb")
        nc.sync.dma_start(out=sin_sb, in_=io["sinb"][:, :])
        rt_sb = singles.tile([ROT, ROT], BF16, tag="rt_sb")
        nc.sync.dma_start(out=rt_sb, in_=io["rt"][:, :])
        masks_sb = singles.tile([128, 4, 512], BF16, tag="masks_sb")
        nc.sync.dma_start(out=masks_sb, in_=io["masks"][:, :, :])
        ones_sb = singles.tile([128, 128], BF16, tag="ones_sb")
        nc.vector.memset(ones_sb, 1.0)
        bqk_sb = singles.tile([128, NBQK], F32, tag="bqk_sb")
        nc.sync.dma_start(out=bqk_sb, in_=io["bqk"].rearrange("(n p) -> p n", p=128))
        bd_sb = singles.tile([128, 4], F32, tag="bd_sb")
        nc.sync.dma_start(out=bd_sb, in_=io["bd"].rearrange("(n p) -> p n", p=128))
        bv_row = singles.tile([1, HPG * 128], F32, tag="bv_row")
        nc.sync.dma_start(out=bv_row, in_=io["bv"][None, :])
        bvB = singles.tile([128, HPG * 128], F32, tag="bvB")
        nc.gpsimd.partition_broadcast(bvB, bv_row)

        wv_sb = []
        wd_sb = []

        def load_wv():
            for kb in range(KB):
                wv_t = wpool.tile([128, 512], BF16, tag=f"wv{kb}", name=f"wv{kb}")
                nc.sync.dma_start(out=wv_t, in_=io["wv"][kb])
                wv_sb.append(wv_t)

        def load_wd():
            for kb in range(KB):
                wd_t = wpool.tile([128, 512], BF16, tag=f"wd{kb}", name=f"wd{kb}")
                nc.sync.dma_start(out=wd_t, in_=io["wd"][kb])
                wd_sb.append(wd_t)

        # warmup collective: the first collective on the CC stream pays a
        # ~40us setup cost; absorb it with a tiny AllGather during phase A
        warm_in = dram.tile([128, 2], BF16, tag="warm_in", name="warm_in")
        warm_out = dram.tile([512, 2], BF16, tag="warm_out", name="warm_out")
        wz = work.tile([128, 2], BF16, tag="wz", bufs=1, name="wz")
        nc.vector.memset(wz, 0.0)
        nc.sync.dma_start(out=warm_in[:, :], in_=wz)
        nc.gpsimd.collective_compute(
            "AllGather",
            mybir.AluOpType.bypass,
            replica_groups=[[0, 1, 2, 3], [4, 5, 6, 7]],
            ins=[warm_in.opt()],
            outs=[warm_out.opt()],
        )

        # ---- DRAM bounce buffers, one per (q-block, head-pair) ----
        # splitting the AllGather in half lets it start as soon as the first
        # two heads of a q-block are done, and halves the tail of the last one
        bounce = {}
        gath = {}
        for j in range(MB):
            for half in range(2):
                bounce[(j, half)] = dram.tile(
                    [256, 512], BF16, tag=f"bounce{j}_{half}", name=f"bounce{j}_{half}"
                )
                gath[(j, half)] = dram.tile(
                    [1024, 512], BF16, tag=f"gath{j}_{half}", name=f"gath{j}_{half}"
                )

        # ---- persistent qkT / V tiles ----
        qkT = {}
        for nb in range(NBQK):
            for mb in range(MB):
                qkT[(nb, mb)] = wpool.tile(
                    [128, 512], BF16, tag=f"qkT_{nb}_{mb}", name=f"qkT_{nb}_{mb}"
                )
        v_sb = []
        for km in range(KB):
            v_sb.append(
                wpool.tile([128, HPG * 128], BF16, tag=f"v_{km}", name=f"v_{km}")
            )

        # ============ phase emitters ============

        def phase_a(mb):
            """projections for m-block mb + RoPE"""
            if mb == 0:
                xbf = xbf0
                load_wv()
            else:
                xbf = []
                for kb in range(KB):
                    xb = work.tile([128, 512], BF16, tag="xbf", bufs=18, name="xb")
                    nc.sync.dma_start(
                        out=xb,
                        in_=io["xT"][
                            128 * kb : 128 * (kb + 1), 512 * mb : 512 * (mb + 1)
                        ],
                    )
                    xbf.append(xb)

            for nb in range(NBQK):
                ps = psum.tile([128, 512], F32, tag="acc", bufs=2, name="ps_qk")
                for kb in range(KB):
                    nc.tensor.matmul(
                        ps,
                        wqk_sb[nb][:, kb, :],
                        xbf[kb],
                        start=(kb == 0),
                        stop=(kb == KB - 1),
                    )
                nc.vector.tensor_scalar_add(
                    out=qkT[(nb, mb)], in0=ps, scalar1=bqk_sb[:, nb : nb + 1]
                )

            for msub in range(4):
                km = 4 * mb + msub
                ps = psum.tile([128, HPG * 128], F32, tag="acc", bufs=2, name="ps_v")
                for kb in range(KB):
                    nc.tensor.matmul(
                        ps,
                        xbf[kb][:, 128 * msub : 128 * (msub + 1)],
                        wv_sb[kb],
                        start=(kb == 0),
                        stop=(kb == KB - 1),
                    )
                nc.vector.tensor_add(out=v_sb[km], in0=ps, in1=bvB)

            # RoPE: a' = a*cos + (Rt.T @ a)*sin on the first 32 partitions
            csl = cos_sb[:, 512 * mb : 512 * (mb + 1)]
            ssl = sin_sb[:, 512 * mb : 512 * (mb + 1)]
            for h in range(HPG):
                for qk in range(2):
                    a = qkT[(2 * h + qk, mb)]
                    ps_r = psum.tile([ROT, 512], F32, tag="score", bufs=3, name="ps_r")
                    nc.tensor.matmul(ps_r, rt_sb, a[0:ROT, :], start=True, stop=True)
                    tq = work.tile([ROT, 512], BF16, tag=f"ropeq{qk}", bufs=1)
                    nc.gpsimd.tensor_mul(out=tq, in0=a[0:ROT, :], in1=csl)
                    ts = work.tile([ROT, 512], BF16, tag=f"ropes{qk}", bufs=1)
                    nc.vector.tensor_mul(out=ts, in0=ps_r, in1=ssl)
                    nc.gpsimd.tensor_add(out=a[0:ROT, :], in0=tq, in1=ts)

        def phase_b(j):
            """attention for q-block j (all heads) + bounce DMA + AllGather"""
            nkm = 4 * j + 4
            for h in range(HPG):
                qt = qkT[(2 * h, j)]

                def mk_u(i):
                    # diagonal blocks only need q-columns >= 128*(i-4j): compute
                    # the triangular remainder, mask only the first 128 columns
                    qoff = max(0, 128 * (i - 4 * j))
                    width = 512 - qoff
                    kt = qkT[(2 * h + 1, i // 4)]
                    ps_s = psum.tile([128, 512], F32, tag="score", bufs=3, name="ps_s")
                    nc.tensor.matmul(
                        ps_s[:, 0:width],
                        kt[:, 128 * (i % 4) : 128 * (i % 4 + 1)],
                        qt[:, qoff:512],
                        start=True,
                        stop=True,
                    )
                    u = work.tile([128, 512], BF16, tag="u", bufs=5, name="u")
                    nc.scalar.activation(
                        out=u[:, 0:width], in_=ps_s[:, 0:width],
                        func=mybir.ActivationFunctionType.Exp, scale=SCALE,
                    )
                    if i >= 4 * j:
                        nc.vector.tensor_mul(
                            out=u[:, 0:128], in0=u[:, 0:128],
                            in1=masks_sb[:, 0, 0:128],
                        )
                    return u, qoff, width

                ps_av = psum.tile([128, 512], F32, tag="av", bufs=3, name="ps_av")
                ps_sum = psum.tile([128, 512], F32, tag="acc", bufs=2, name="ps_sum")
                pipe = [mk_u(0)]
                if nkm > 1:
                    pipe.append(mk_u(1))
                for i in range(nkm):
                    u, qoff, width = pipe.pop(0)
                    if i + 2 < nkm:
                        pipe.append(mk_u(i + 2))
                    nc.tensor.matmul(
                        ps_av[:, qoff:512],
                        v_sb[i][:, 128 * h : 128 * (h + 1)],
                        u[:, 0:width],
                        start=(i == 0),
                        stop=(i == nkm - 1),
                    )
                    nc.tensor.matmul(
                        ps_sum[:, qoff:512],
                        ones_sb,
                        u[:, 0:width],
                        start=(i == 0),
                        stop=(i == nkm - 1),
                    )
                # free the sum bank fast (533ns copy), then slow reciprocal
                sums = work.tile([128, 512], F32, tag="sums", bufs=2, name="sums")
                nc.vector.tensor_copy(out=sums, in_=ps_sum)
                recipB = work.tile([128, 512], F32, tag="recipB", bufs=2, name="recipB")
                nc.vector.reciprocal(out=recipB, in_=sums)
                attn_t = work.tile([128, 512], BF16, tag="attnT", bufs=6, name="attn_t")
                nc.vector.tensor_mul(out=attn_t, in0=ps_av, in1=recipB)
                nc.sync.dma_start(
                    out=bounce[(j, h // 2)][128 * (h % 2) : 128 * (h % 2 + 1), :],
                    in_=attn_t,
                )
                if h % 2 == 1:
                    nc.gpsimd.collective_compute(
                        "AllGather",
                        mybir.AluOpType.bypass,
                        replica_groups=[[0, 1, 2, 3], [4, 5, 6, 7]],
                        ins=[bounce[(j, h // 2)].opt()],
                        outs=[gath[(j, h // 2)].opt()],
                    )

        def phase_c(j):
            """dense for q-block j"""
            # gath half layout: rank r, local head l in {0,1}, gives row block
            # i = 2r + l  <->  hidden block 4r + 2*half + l
            ga = []  # list of (hd_block, tile), in arrival order (half-major)
            for half in range(2):
                for i in range(8):
                    hd = 4 * (i // 2) + 2 * half + (i % 2)
                    g_t = work.tile([128, 512], BF16, tag="ga", bufs=18, name="ga")
                    nc.sync.dma_start(
                        out=g_t, in_=gath[(j, half)][128 * i : 128 * (i + 1), :]
                    )
                    ga.append((hd, g_t))
            for ob in range(4):
                ps_d = psum.tile([128, 512], F32, tag="acc", bufs=2, name="ps_d")
                for idx, (hd, g_t) in enumerate(ga):
                    nc.tensor.matmul(
                        ps_d,
                        wd_sb[hd][:, 128 * ob : 128 * (ob + 1)],
                        g_t,
                        start=(idx == 0),
                        stop=(idx == KB - 1),
                    )
                o_sb = work.tile([128, 512], F32, tag="o_sb", bufs=3, name="o_sb")
                nc.vector.tensor_scalar_add(
                    out=o_sb, in0=ps_d, scalar1=bd_sb[:, ob : ob + 1]
                )
                nc.sync.dma_start(
                    out=io["outT"][128 * ob : 128 * (ob + 1), 512 * j : 512 * (j + 1)],
                    in_=o_sb,
                )

        # ============ emission order ============
        # interleave so every consumer is emitted >=1 full phase after its
        # producer: PE never head-of-line blocks on ACT/DVE/collective.
        phase_a(0)
        phase_a(1)
        phase_b(0)
        load_wd()
        phase_a(2)
        phase_b(1)
        phase_a(3)
        phase_b(2)
        phase_c(0)
        phase_c(1)
        phase_b(3)
        phase_c(2)
        phase_c(3)


def _prep_inputs(x, position_ids, Wqkv, bqkv, Wdense, bdense):
    """Host-side sharding + bf16 pre-cast + weight re-layout."""
    bf16 = ml_dtypes.bfloat16
    inv_freq = 1.0 / (BASE ** (np.arange(0, ROT, 2, dtype=np.float32) / ROT))

    # diagonal-block masks: mask[p][kk, qq] = 1 if qq >= kk + 128*p
    kk = np.arange(128)[:, None]
    qq = np.arange(512)[None, :]
    masks = np.stack(
        [(qq >= kk + 128 * p) for p in range(4)], axis=1
    ).astype(bf16)  # [128, 4, 512]

    R = np.zeros((ROT, ROT), np.float32)
    R[np.arange(16), np.arange(16) + 16] = -1.0
    R[np.arange(16) + 16, np.arange(16)] = 1.0
    rt = np.ascontiguousarray(R.T).astype(bf16)

    in_maps = []
    for c in range(NCORES):
        b, g = divmod(c, G)
        heads = range(HPG * g, HPG * (g + 1))
        xTb = np.ascontiguousarray(x[b].T).astype(bf16)  # [HID, S]
        wqk = np.concatenate(
            [Wqkv[:, 384 * h : 384 * h + 256] for h in heads], axis=1
        )  # [HID, 1024]
        # -> [nb, p, kb*128+n]
        wqk = np.ascontiguousarray(
            wqk.reshape(KB, 128, NBQK, 128).transpose(2, 1, 0, 3).reshape(
                NBQK, 128, KB * 128
            )
        ).astype(bf16)
        wv = np.concatenate(
            [Wqkv[:, 384 * h + 256 : 384 * h + 384] for h in heads], axis=1
        ).reshape(KB, 128, 512).astype(bf16)
        bqk = np.concatenate(
            [bqkv[384 * h : 384 * h + 256] for h in heads]
        ).astype(np.float32)
        bv = np.concatenate(
            [bqkv[384 * h + 256 : 384 * h + 384] for h in heads]
        ).astype(np.float32)
        wd = np.ascontiguousarray(Wdense[:, 512 * g : 512 * (g + 1)]).reshape(
            KB, 128, 512
        ).astype(bf16)
        bd = np.ascontiguousarray(bdense[512 * g : 512 * (g + 1)]).astype(np.float32)
        ang = np.outer(inv_freq, position_ids[b].astype(np.float32))  # [16, S]
        cosE = np.concatenate([np.cos(ang)] * 2, axis=0)  # [32, S]
        sinE = np.concatenate([np.sin(ang)] * 2, axis=0)
        in_maps.append(
            {
                "xT": xTb,
                "wqk": wqk,
                "wv": wv,
                "bqk": bqk,
                "bv": bv,
                "wd": wd,
                "bd": bd,
                "cosb": cosE.astype(bf16),
                "sinb": sinE.astype(bf16),
                "rt": rt,
                "masks": masks,
            }
        )
    return in_maps


def _run(in_maps, trace=False):
    if "nc" not in _CACHE:
        _CACHE["nc"] = _build_nc()
    nc = _CACHE["nc"]
    res = bass_utils.run_bass_kernel_spmd(
        nc, in_maps, core_ids=list(range(NCORES)), trace=trace
    )
    return res


def kernel(x, position_ids, attention_mask, Wqkv, bqkv, Wdense, bdense,
           _trace=False, _return_results=False):
    x = np.asarray(x, dtype=np.float32)
    position_ids = np.asarray(position_ids)
    Wqkv = np.asarray(Wqkv, dtype=np.float32)
    bqkv = np.asarray(bqkv, dtype=np.float32)
    Wdense = np.asarray(Wdense, dtype=np.float32)
    bdense = np.asarray(bdense, dtype=np.float32)

    in_maps = _prep_inputs(x, position_ids, Wqkv, bqkv, Wdense, bdense)
    res = _run(in_maps, trace=_trace)

    y = np.empty((B, S, HID), dtype=np.float32)
    for c in range(NCORES):
        b, g = divmod(c, G)
        y[b, :, 512 * g : 512 * (g + 1)] = res.results[c]["outT"].T
    if _return_results:
        return y, res
    return y


# revision 25
# speedup vs baseline: 1.1069x; 1.1069x over previous
"""Causal self-attention (dense transformer block) on 8 Trainium2 NeuronCores.

Problem (hardcoded): B=2, S=2048, HID=2048, NH=16, HS=128, ROT=32 (partial
rotary), causal additive mask, f32 I/O.

Sharding: core c = b*4 + g handles batch b and head-group g (4 heads).
 - Phase A (projections): qkT = (x @ Wqk_shard)^T computed from host-provided
   xT (x[b] transposed, pre-cast bf16) so the contraction dim lands on
   partitions.  Q,K are produced transposed ([d, s]); V natural ([s, d]).
 - RoPE in-place on the first 32 dims of each head of Q,K; rotate_half is a
   32x32 sign-permutation matmul (engine partition ops must be 32-aligned).
 - Phase B (attention): S^T tiles [128 k, 512 q] on PE, exp on ACT (no max
   subtraction; scores are O(5)), causality structural (upper blocks skipped,
   diagonal blocks masked after exp), AV and row-sum accumulated on PE
   (row-sum via all-ones 128x128 stationary => result replicated across
   partitions, no broadcast needed).
 - Per-q-block AllGather (groups of 4 = same batch) overlapped with later
   compute phases; dense (column-parallel Wdense) per q-block right after.
 - Host reassembles the 8 [512, 2048] f32 transposed output slices.
"""

import numpy as np
import ml_dtypes

import concourse.bass as bass
import concourse.bacc as bacc
import concourse.mybir as mybir
import concourse.tile as tile
from concourse import bass_utils

B, S, HID = 2, 2048, 2048
NH = 16
HS = 128
ROT = 32
BASE = 10000.0
G = 4            # head-groups (4 heads each)
HPG = NH // G    # heads per core = 4
NCORES = 8

MB = 4           # m (seq) blocks of 512
KB = 16          # contraction blocks of 128
NBQK = 2 * HPG   # qk feature blocks of 128 (q,k interleaved per head)
F32 = mybir.dt.float32
BF16 = mybir.dt.bfloat16
SCALE = 1.0 / float(np.sqrt(np.float32(HS)))

_CACHE = {}


def _build_nc():
    nc = bacc.Bacc(
        "TRN2", target_bir_lowering=False, debug=False, num_devices=NCORES
    )

    io = {
        "xT": nc.dram_tensor("xT", [HID, S], BF16, kind="ExternalInput"),
        # wqk host-layout: [nb, p, kb*128+n] so each nb is one contiguous DMA
        "wqk": nc.dram_tensor("wqk", [NBQK, 128, KB * 128], BF16, kind="ExternalInput"),
        "wv": nc.dram_tensor("wv", [KB, 128, 512], BF16, kind="ExternalInput"),
        "wd": nc.dram_tensor("wd", [KB, 128, 512], BF16, kind="ExternalInput"),
        "bqk": nc.dram_tensor("bqk", [NBQK * 128], F32, kind="ExternalInput"),
        "bv": nc.dram_tensor("bv", [HPG * 128], F32, kind="ExternalInput"),
        "bd": nc.dram_tensor("bd", [512], F32, kind="ExternalInput"),
        "cosb": nc.dram_tensor("cosb", [ROT, S], BF16, kind="ExternalInput"),
        "sinb": nc.dram_tensor("sinb", [ROT, S], BF16, kind="ExternalInput"),
        "rt": nc.dram_tensor("rt", [ROT, ROT], BF16, kind="ExternalInput"),
        "masks": nc.dram_tensor("masks", [128, 4, 512], BF16, kind="ExternalInput"),
        "outT": nc.dram_tensor("outT", [512, S], F32, kind="ExternalOutput"),
    }

    with tile.TileContext(nc) as tc:
        _kernel_body(tc, io)

    nc.compile()
    return nc


def _kernel_body(tc, io):
    nc = tc.nc
    from contextlib import ExitStack

    with ExitStack() as ctx:
        singles = ctx.enter_context(tc.tile_pool(name="singles", bufs=1))
        wpool = ctx.enter_context(tc.tile_pool(name="wpool", bufs=1))
        work = ctx.enter_context(tc.tile_pool(name="work", bufs=2))
        psum = ctx.enter_context(tc.tile_pool(name="psum", bufs=2, space="PSUM"))
        dram = ctx.enter_context(tc.tile_pool(name="dram", bufs=1, space="DRAM"))

        # ---- weights for the very first matmuls, in load order ----
        # (first qk weight block, then the first m-block of x, then the rest:
        # the first PE matmul needs only wqk[0] + xbf(0))
        wqk_sb = []
        for nb in range(NBQK):
            w_t = wpool.tile([128, KB, 128], BF16, tag=f"wqk{nb}", name=f"wqk{nb}")
            nc.sync.dma_start(out=w_t, in_=io["wqk"][nb])
            wqk_sb.append(w_t)
            if nb == 0:
                xbf0 = []
                for kb in range(KB):
                    xb = work.tile([128, 512], BF16, tag="xbf", bufs=18, name="xb")
                    nc.sync.dma_start(
                        out=xb, in_=io["xT"][128 * kb : 128 * (kb + 1), 0:512]
                    )
                    xbf0.append(xb)

        # ---- constants ----
        cos_sb = singles.tile([ROT, S], BF16, tag="cos_sb")
        nc.sync.dma_start(out=cos_sb, in_=io["cosb"][:, :])
        sin_sb = singles.tile([ROT, S], BF16, tag="sin_sb")
        nc.sync.dma_start(out=sin_sb, in_=io["sinb"][:, :])
        rt_sb = singles.tile([ROT, ROT], BF16, tag="rt_sb")
        nc.sync.dma_start(out=rt_sb, in_=io["rt"][:, :])
        masks_sb = singles.tile([128, 4, 512], BF16, tag="masks_sb")
        nc.sync.dma_start(out=masks_sb, in_=io["masks"][:, :, :])
        ones_sb = singles.tile([128, 128], BF16, tag="ones_sb")
        nc.vector.memset(ones_sb, 1.0)
        bqk_sb = singles.tile([128, NBQK], F32, tag="bqk_sb")
        nc.sync.dma_start(out=bqk_sb, in_=io["bqk"].rearrange("(n p) -> p n", p=128))
        bd_sb = singles.tile([128, 4], F32, tag="bd_sb")
        nc.sync.dma_start(out=bd_sb, in_=io["bd"].rearrange("(n p) -> p n", p=128))
        bv_row = singles.tile([1, HPG * 128], F32, tag="bv_row")
        nc.sync.dma_start(out=bv_row, in_=io["bv"][None, :])
        bvB = singles.tile([128, HPG * 128], F32, tag="bvB")
        nc.gpsimd.partition_broadcast(bvB, bv_row)

        wv_sb = []
        wd_sb = []

        def load_wv():
            for kb in range(KB):
                wv_t = wpool.tile([128, 512], BF16, tag=f"wv{kb}", name=f"wv{kb}")
                nc.sync.dma_start(out=wv_t, in_=io["wv"][kb])
                wv_sb.append(wv_t)

        def load_wd():
            for kb in range(KB):
                wd_t = wpool.tile([128, 512], BF16, tag=f"wd{kb}", name=f"wd{kb}")
                nc.sync.dma_start(out=wd_t, in_=io["wd"][kb])
                wd_sb.append(wd_t)

        # warmup collective: the first collective on the CC stream pays a
        # ~40us setup cost; absorb it with a tiny AllGather during phase A
        warm_in = dram.tile([128, 2], BF16, tag="warm_in", name="warm_in")
        warm_out = dram.tile([512, 2], BF16, tag="warm_out", name="warm_out")
        wz = work.tile([128, 2], BF16, tag="wz", bufs=1, name="wz")
        nc.vector.memset(wz, 0.0)
        nc.sync.dma_start(out=warm_in[:, :], in_=wz)
        nc.gpsimd.collective_compute(
            "AllGather",
            mybir.AluOpType.bypass,
            replica_groups=[[0, 1, 2, 3], [4, 5, 6, 7]],
            ins=[warm_in.opt()],
            outs=[warm_out.opt()],
        )

        # ---- DRAM bounce buffers, one per (q-block, head-pair) ----
        # splitting the AllGather in half lets it start as soon as the first
        # two heads of a q-block are done, and halves the tail of the last one
        bounce = {}
        gath = {}
        for j in range(MB):
            for half in range(2):
                bounce[(j, half)] = dram.tile(
                    [256, 512], BF16, tag=f"bounce{j}_{half}", name=f"bounce{j}_{half}"
                )
                gath[(j, half)] = dram.tile(
                    [1024, 512], BF16, tag=f"gath{j}_{half}", name=f"gath{j}_{half}"
                )

        # ---- persistent qkT / V tiles ----
        qkT = {}
        for nb in range(NBQK):
            for mb in range(MB):
                qkT[(nb, mb)] = wpool.tile(
                    [128, 512], BF16, tag=f"qkT_{nb}_{mb}", name=f"qkT_{nb}_{mb}"
                )
        v_sb = []
        for km in range(KB):
            v_sb.append(
                wpool.tile([128, HPG * 128], BF16, tag=f"v_{km}", name=f"v_{km}")
            )

        # ============ phase emitters ============

        def phase_a(mb):
            """projections for m-block mb + RoPE"""
            if mb == 0:
                xbf = xbf0
                load_wv()
            else:
                xbf = []
                for kb in range(KB):
                    xb = work.tile([128, 512], BF16, tag="xbf", bufs=18, name="xb")
                    nc.sync.dma_start(
                        out=xb,
                        in_=io["xT"][
                            128 * kb : 128 * (kb + 1), 512 * mb : 512 * (mb + 1)
                        ],
                    )
                    xbf.append(xb)

            for nb in range(NBQK):
                ps = psum.tile([128, 512], F32, tag="acc", bufs=2, name="ps_qk")
                for kb in range(KB):
                    nc.tensor.matmul(
                        ps,
                        wqk_sb[nb][:, kb, :],
                        xbf[kb],
                        start=(kb == 0),
                        stop=(kb == KB - 1),
                    )
                nc.vector.tensor_scalar_add(
                    out=qkT[(nb, mb)], in0=ps, scalar1=bqk_sb[:, nb : nb + 1]
                )

            for msub in range(4):
                km = 4 * mb + msub
                ps = psum.tile([128, HPG * 128], F32, tag="acc", bufs=2, name="ps_v")
                for kb in range(KB):
                    nc.tensor.matmul(
                        ps,
                        xbf[kb][:, 128 * msub : 128 * (msub + 1)],
                        wv_sb[kb],
                        start=(kb == 0),
                        stop=(kb == KB - 1),
                    )
                nc.vector.tensor_add(out=v_sb[km], in0=ps, in1=bvB)

            # RoPE: a' = a*cos + (Rt.T @ a)*sin on the first 32 partitions
            csl = cos_sb[:, 512 * mb : 512 * (mb + 1)]
            ssl = sin_sb[:, 512 * mb : 512 * (mb + 1)]
            for h in range(HPG):
                for qk in range(2):
                    a = qkT[(2 * h + qk, mb)]
                    ps_r = psum.tile([ROT, 512], F32, tag="score", bufs=3, name="ps_r")
                    nc.tensor.matmul(ps_r, rt_sb, a[0:ROT, :], start=True, stop=True)
                    tq = work.tile([ROT, 512], BF16, tag=f"ropeq{qk}", bufs=1)
                    nc.gpsimd.tensor_mul(out=tq, in0=a[0:ROT, :], in1=csl)
                    ts = work.tile([ROT, 512], BF16, tag=f"ropes{qk}", bufs=1)
                    nc.vector.tensor_mul(out=ts, in0=ps_r, in1=ssl)
                    nc.gpsimd.tensor_add(out=a[0:ROT, :], in0=tq, in1=ts)

        def phase_b(j):
            """attention for q-block j (all heads) + bounce DMA + AllGather"""
            nkm = 4 * j + 4
            for h in range(HPG):
                qt = qkT[(2 * h, j)]

                def mk_u(i):
                    # diagonal blocks only need q-columns >= 128*(i-4j): compute
                    # the triangular remainder, mask only the first 128 columns
                    qoff = max(0, 128 * (i - 4 * j))
                    width = 512 - qoff
                    kt = qkT[(2 * h + 1, i // 4)]
                    ps_s = psum.tile([128, 512], F32, tag="score", bufs=3, name="ps_s")
                    nc.tensor.matmul(
                        ps_s[:, 0:width],
                        kt[:, 128 * (i % 4) : 128 * (i % 4 + 1)],
                        qt[:, qoff:512],
                        start=True,
                        stop=True,
                    )
                    u = work.tile([128, 512], BF16, tag="u", bufs=5, name="u")
                    nc.scalar.activation(
                        out=u[:, 0:width], in_=ps_s[:, 0:width],
                        func=mybir.ActivationFunctionType.Exp, scale=SCALE,
                    )
                    if i >= 4 * j:
                        nc.vector.tensor_mul(
                            out=u[:, 0:128], in0=u[:, 0:128],
                            in1=masks_sb[:, 0, 0:128],
                        )
                    return u, qoff, width

                ps_av = psum.tile([128, 512], F32, tag="av", bufs=2, name="ps_av")
                ps_sum = psum.tile([128, 512], F32, tag="sum", bufs=1, name="ps_sum")
                pipe = [mk_u(0)]
                if nkm > 1:
                    pipe.append(mk_u(1))
                for i in range(nkm):
                    u, qoff, width = pipe.pop(0)
                    if i + 2 < nkm:
                        pipe.append(mk_u(i + 2))
                    nc.tensor.matmul(
                        ps_av[:, qoff:512],
                        v_sb[i][:, 128 * h : 128 * (h + 1)],
                        u[:, 0:width],
                        start=(i == 0),
                        stop=(i == nkm - 1),
                    )
                    nc.tensor.matmul(
                        ps_sum[:, qoff:512],
                        ones_sb,
                        u[:, 0:width],
                        start=(i == 0),
                        stop=(i == nkm - 1),
                    )
                # free the sum bank fast (533ns copy), then slow reciprocal
                sums = work.tile([128, 512], F32, tag="sums", bufs=2, name="sums")
                nc.vector.tensor_copy(out=sums, in_=ps_sum)
                recipB = work.tile([128, 512], F32, tag="recipB", bufs=2, name="recipB")
                nc.vector.reciprocal(out=recipB, in_=sums)
                attn_t = work.tile([128, 512], BF16, tag="attnT", bufs=6, name="attn_t")
                nc.vector.tensor_mul(out=attn_t, in0=ps_av, in1=recipB)
                nc.sync.dma_start(
                    out=bounce[(j, h // 2)][128 * (h % 2) : 128 * (h % 2 + 1), :],
                    in_=attn_t,
                )
                if h % 2 == 1:
                    nc.gpsimd.collective_compute(
                        "AllGather",
                        mybir.AluOpType.bypass,
                        replica_groups=[[0, 1, 2, 3], [4, 5, 6, 7]],
                        ins=[bounce[(j, h // 2)].opt()],
                        outs=[gath[(j, h // 2)].opt()],
                    )

        def phase_c(j):
            """dense for q-block j"""
            # gath half layout: rank r, local head l in {0,1}, gives row block
            # i = 2r + l  <->  hidden block 4r + 2*half + l
            ga = []  # list of (hd_block, tile), in arrival order (half-major)
            for half in range(2):
                for i in range(8):
                    hd = 4 * (i // 2) + 2 * half + (i % 2)
                    g_t = work.tile([128, 512], BF16, tag="ga", bufs=18, name="ga")
                    nc.sync.dma_start(
                        out=g_t, in_=gath[(j, half)][128 * i : 128 * (i + 1), :]
                    )
                    ga.append((hd, g_t))
            for ob in range(4):
                ps_d = psum.tile([128, 512], F32, tag="acc", bufs=2, name="ps_d")
                for idx, (hd, g_t) in enumerate(ga):
                    nc.tensor.matmul(
                        ps_d,
                        wd_sb[hd][:, 128 * ob : 128 * (ob + 1)],
                        g_t,
                        start=(idx == 0),
                        stop=(idx == KB - 1),
                    )
                o_sb = work.tile([128, 512], F32, tag="o_sb", bufs=3, name="o_sb")
                nc.vector.tensor_scalar_add(
                    out=o_sb, in0=ps_d, scalar1=bd_sb[:, ob : ob + 1]
                )
                nc.sync.dma_start(
                    out=io["outT"][128 * ob : 128 * (ob + 1), 512 * j : 512 * (j + 1)],
                    in_=o_sb,
                )

        # ============ emission order ============
        # interleave so every consumer is emitted >=1 full phase after its
        # producer: PE never head-of-line blocks on ACT/DVE/collective.
        phase_a(0)
        phase_a(1)
        phase_b(0)
        load_wd()
        phase_a(2)
        phase_b(1)
        phase_a(3)
        phase_b(2)
        phase_c(0)
        phase_c(1)
        phase_b(3)
        phase_c(2)
        phase_c(3)


def _prep_inputs(x, position_ids, Wqkv, bqkv, Wdense, bdense):
    """Host-side sharding + bf16 pre-cast + weight re-layout."""
    bf16 = ml_dtypes.bfloat16
    inv_freq = 1.0 / (BASE ** (np.arange(0, ROT, 2, dtype=np.float32) / ROT))

    # diagonal-block masks: mask[p][kk, qq] = 1 if qq >= kk + 128*p
    kk = np.arange(128)[:, None]
    qq = np.arange(512)[None, :]
    masks = np.stack(
        [(qq >= kk + 128 * p) for p in range(4)], axis=1
    ).astype(bf16)  # [128, 4, 512]

    R = np.zeros((ROT, ROT), np.float32)
    R[np.arange(16), np.arange(16) + 16] = -1.0
    R[np.arange(16) + 16, np.arange(16)] = 1.0
    rt = np.ascontiguousarray(R.T).astype(bf16)

    in_maps = []
    for c in range(NCORES):
        b, g = divmod(c, G)
        heads = range(HPG * g, HPG * (g + 1))
        xTb = np.ascontiguousarray(x[b].T).astype(bf16)  # [HID, S]
        wqk = np.concatenate(
            [Wqkv[:, 384 * h : 384 * h + 256] for h in heads], axis=1
        )  # [HID, 1024]
        # -> [nb, p, kb*128+n]
        wqk = np.ascontiguousarray(
            wqk.reshape(KB, 128, NBQK, 128).transpose(2, 1, 0, 3).reshape(
                NBQK, 128, KB * 128
            )
        ).astype(bf16)
        wv = np.concatenate(
            [Wqkv[:, 384 * h + 256 : 384 * h + 384] for h in heads], axis=1
        ).reshape(KB, 128, 512).astype(bf16)
        bqk = np.concatenate(
            [bqkv[384 * h : 384 * h + 256] for h in heads]
        ).astype(np.float32)
        bv = np.concatenate(
            [bqkv[384 * h + 256 : 384 * h + 384] for h in heads]
        ).astype(np.float32)
        wd = np.ascontiguousarray(Wdense[:, 512 * g : 512 * (g + 1)]).reshape(
            KB, 128, 512
        ).astype(bf16)
        bd = np.ascontiguousarray(bdense[512 * g : 512 * (g + 1)]).astype(np.float32)
        ang = np.outer(inv_freq, position_ids[b].astype(np.float32))  # [16, S]
        cosE = np.concatenate([np.cos(ang)] * 2, axis=0)  # [32, S]
        sinE = np.concatenate([np.sin(ang)] * 2, axis=0)
        in_maps.append(
            {
                "xT": xTb,
                "wqk": wqk,
                "wv": wv,
                "bqk": bqk,
                "bv": bv,
                "wd": wd,
                "bd": bd,
                "cosb": cosE.astype(bf16),
                "sinb": sinE.astype(bf16),
                "rt": rt,
                "masks": masks,
            }
        )
    return in_maps


def _run(in_maps, trace=False):
    if "nc" not in _CACHE:
        _CACHE["nc"] = _build_nc()
    nc = _CACHE["nc"]
    res = bass_utils.run_bass_kernel_spmd(
        nc, in_maps, core_ids=list(range(NCORES)), trace=trace
    )
    return res


def kernel(x, position_ids, attention_mask, Wqkv, bqkv, Wdense, bdense,
           _trace=False, _return_results=False):
    x = np.asarray(x, dtype=np.float32)
    position_ids = np.asarray(position_ids)
    Wqkv = np.asarray(Wqkv, dtype=np.float32)
    bqkv = np.asarray(bqkv, dtype=np.float32)
    Wdense = np.asarray(Wdense, dtype=np.float32)
    bdense = np.asarray(bdense, dtype=np.float32)

    in_maps = _prep_inputs(x, position_ids, Wqkv, bqkv, Wdense, bdense)
    res = _run(in_maps, trace=_trace)

    y = np.empty((B, S, HID), dtype=np.float32)
    for c in range(NCORES):
        b, g = divmod(c, G)
        y[b, :, 512 * g : 512 * (g + 1)] = res.results[c]["outT"].T
    if _return_results:
        return y, res
    return y


# revision 28
# speedup vs baseline: 1.1244x; 1.0158x over previous
"""Causal self-attention (dense transformer block) on 8 Trainium2 NeuronCores.

Problem (hardcoded): B=2, S=2048, HID=2048, NH=16, HS=128, ROT=32 (partial
rotary), causal additive mask, f32 I/O.

Sharding: core c = b*4 + g handles batch b and head-group g (4 heads).
 - Phase A (projections): qkT = (x @ Wqk_shard)^T computed from host-provided
   xT (x[b] transposed, pre-cast bf16) so the contraction dim lands on
   partitions.  Q,K are produced transposed ([d, s]); V natural ([s, d]).
 - RoPE in-place on the first 32 dims of each head of Q,K; rotate_half is a
   32x32 sign-permutation matmul (engine partition ops must be 32-aligned).
 - Phase B (attention): S^T tiles [128 k, 512 q] on PE, exp on ACT (no max
   subtraction; scores are O(5)), causality structural (upper blocks skipped,
   diagonal blocks masked after exp), AV and row-sum accumulated on PE
   (row-sum via all-ones 128x128 stationary => result replicated across
   partitions, no broadcast needed).
 - Per-q-block AllGather (groups of 4 = same batch) overlapped with later
   compute phases; dense (column-parallel Wdense) per q-block right after.
 - Host reassembles the 8 [512, 2048] f32 transposed output slices.
"""

import numpy as np
import ml_dtypes

import concourse.bass as bass
import concourse.bacc as bacc
import concourse.mybir as mybir
import concourse.tile as tile
from concourse import bass_utils

B, S, HID = 2, 2048, 2048
NH = 16
HS = 128
ROT = 32
BASE = 10000.0
G = 4            # head-groups (4 heads each)
HPG = NH // G    # heads per core = 4
NCORES = 8

MB = 4           # m (seq) blocks of 512
KB = 16          # contraction blocks of 128
NBQK = 2 * HPG   # qk feature blocks of 128 (q,k interleaved per head)
F32 = mybir.dt.float32
BF16 = mybir.dt.bfloat16
SCALE = 1.0 / float(np.sqrt(np.float32(HS)))

_CACHE = {}


def _build_nc():
    nc = bacc.Bacc(
        "TRN2", target_bir_lowering=False, debug=False, num_devices=NCORES
    )

    io = {
        "xT": nc.dram_tensor("xT", [HID, S], BF16, kind="ExternalInput"),
        # wqk host-layout: [nb, p, kb*128+n] so each nb is one contiguous DMA
        "wqk": nc.dram_tensor("wqk", [NBQK, 128, KB * 128], BF16, kind="ExternalInput"),
        "wv": nc.dram_tensor("wv", [KB, 128, 512], BF16, kind="ExternalInput"),
        "wd": nc.dram_tensor("wd", [KB, 128, 512], BF16, kind="ExternalInput"),
        "bqk": nc.dram_tensor("bqk", [NBQK * 128], F32, kind="ExternalInput"),
        "bv": nc.dram_tensor("bv", [HPG * 128], F32, kind="ExternalInput"),
        "bd": nc.dram_tensor("bd", [512], F32, kind="ExternalInput"),
        "cosb": nc.dram_tensor("cosb", [ROT, S], BF16, kind="ExternalInput"),
        "sinb": nc.dram_tensor("sinb", [ROT, S], BF16, kind="ExternalInput"),
        "rt": nc.dram_tensor("rt", [ROT, ROT], BF16, kind="ExternalInput"),
        "masks": nc.dram_tensor("masks", [128, 4, 512], BF16, kind="ExternalInput"),
        "outT": nc.dram_tensor("outT", [512, S], F32, kind="ExternalOutput"),
    }

    with tile.TileContext(nc) as tc:
        _kernel_body(tc, io)

    nc.compile()
    return nc


def _kernel_body(tc, io):
    nc = tc.nc
    from contextlib import ExitStack

    with ExitStack() as ctx:
        singles = ctx.enter_context(tc.tile_pool(name="singles", bufs=1))
        wpool = ctx.enter_context(tc.tile_pool(name="wpool", bufs=1))
        work = ctx.enter_context(tc.tile_pool(name="work", bufs=2))
        psum = ctx.enter_context(tc.tile_pool(name="psum", bufs=2, space="PSUM"))
        dram = ctx.enter_context(tc.tile_pool(name="dram", bufs=1, space="DRAM"))

        # ---- weights for the very first matmuls, in load order ----
        # (first qk weight block, then the first m-block of x, then the rest:
        # the first PE matmul needs only wqk[0] + xbf(0))
        wqk_sb = []
        for nb in range(NBQK):
            w_t = wpool.tile([128, KB, 128], BF16, tag=f"wqk{nb}", name=f"wqk{nb}")
            nc.sync.dma_start(out=w_t, in_=io["wqk"][nb])
            wqk_sb.append(w_t)
            if nb == 0:
                xbf0 = []
                for kb in range(KB):
                    xb = work.tile([128, 512], BF16, tag="xbf", bufs=32, name="xb")
                    nc.sync.dma_start(
                        out=xb, in_=io["xT"][128 * kb : 128 * (kb + 1), 0:512]
                    )
                    xbf0.append(xb)

        # ---- constants ----
        cos_sb = singles.tile([ROT, S], BF16, tag="cos_sb")
        nc.sync.dma_start(out=cos_sb, in_=io["cosb"][:, :])
        sin_sb = singles.tile([ROT, S], BF16, tag="sin_sb")
        nc.sync.dma_start(out=sin_sb, in_=io["sinb"][:, :])
        rt_sb = singles.tile([ROT, ROT], BF16, tag="rt_sb")
        nc.sync.dma_start(out=rt_sb, in_=io["rt"][:, :])
        masks_sb = singles.tile([128, 4, 512], BF16, tag="masks_sb")
        nc.sync.dma_start(out=masks_sb, in_=io["masks"][:, :, :])
        ones_sb = singles.tile([128, 128], BF16, tag="ones_sb")
        nc.vector.memset(ones_sb, 1.0)
        bqk_sb = singles.tile([128, NBQK], F32, tag="bqk_sb")
        nc.sync.dma_start(out=bqk_sb, in_=io["bqk"].rearrange("(n p) -> p n", p=128))
        bd_sb = singles.tile([128, 4], F32, tag="bd_sb")
        nc.sync.dma_start(out=bd_sb, in_=io["bd"].rearrange("(n p) -> p n", p=128))
        bv_row = singles.tile([1, HPG * 128], F32, tag="bv_row")
        nc.sync.dma_start(out=bv_row, in_=io["bv"][None, :])
        bvB = singles.tile([128, HPG * 128], F32, tag="bvB")
        nc.gpsimd.partition_broadcast(bvB, bv_row)

        wv_sb = []
        wd_sb = []

        def load_wv():
            for kb in range(KB):
                wv_t = wpool.tile([128, 512], BF16, tag=f"wv{kb}", name=f"wv{kb}")
                nc.sync.dma_start(out=wv_t, in_=io["wv"][kb])
                wv_sb.append(wv_t)

        def load_wd():
            for kb in range(KB):
                wd_t = wpool.tile([128, 512], BF16, tag=f"wd{kb}", name=f"wd{kb}")
                nc.sync.dma_start(out=wd_t, in_=io["wd"][kb])
                wd_sb.append(wd_t)

        # ---- DRAM bounce buffers, one per (q-block, head-pair) ----
        # splitting the AllGather in half lets it start as soon as the first
        # two heads of a q-block are done, and halves the tail of the last one
        bounce = {}
        gath = {}
        for j in range(MB):
            for half in range(2):
                bounce[(j, half)] = dram.tile(
                    [256, 512], BF16, tag=f"bounce{j}_{half}", name=f"bounce{j}_{half}"
                )
                gath[(j, half)] = dram.tile(
                    [1024, 512], BF16, tag=f"gath{j}_{half}", name=f"gath{j}_{half}"
                )

        # ---- persistent qkT / V tiles ----
        qkT = {}
        for nb in range(NBQK):
            for mb in range(MB):
                qkT[(nb, mb)] = wpool.tile(
                    [128, 512], BF16, tag=f"qkT_{nb}_{mb}", name=f"qkT_{nb}_{mb}"
                )
        v_sb = []
        for km in range(KB):
            v_sb.append(
                wpool.tile([128, HPG * 128], BF16, tag=f"v_{km}", name=f"v_{km}")
            )

        # ============ phase emitters ============

        def phase_a(mb):
            """projections for m-block mb + RoPE"""
            if mb == 0:
                xbf = xbf0
                load_wv()
            else:
                xbf = []
                for kb in range(KB):
                    xb = work.tile([128, 512], BF16, tag="xbf", bufs=32, name="xb")
                    nc.sync.dma_start(
                        out=xb,
                        in_=io["xT"][
                            128 * kb : 128 * (kb + 1), 512 * mb : 512 * (mb + 1)
                        ],
                    )
                    xbf.append(xb)

            for nb in range(NBQK):
                ps = psum.tile([128, 512], F32, tag="acc", bufs=2, name="ps_qk")
                for kb in range(KB):
                    nc.tensor.matmul(
                        ps,
                        wqk_sb[nb][:, kb, :],
                        xbf[kb],
                        start=(kb == 0),
                        stop=(kb == KB - 1),
                    )
                nc.vector.tensor_scalar_add(
                    out=qkT[(nb, mb)], in0=ps, scalar1=bqk_sb[:, nb : nb + 1]
                )

            for msub in range(4):
                km = 4 * mb + msub
                ps = psum.tile([128, HPG * 128], F32, tag="acc", bufs=2, name="ps_v")
                for kb in range(KB):
                    nc.tensor.matmul(
                        ps,
                        xbf[kb][:, 128 * msub : 128 * (msub + 1)],
                        wv_sb[kb],
                        start=(kb == 0),
                        stop=(kb == KB - 1),
                    )
                nc.vector.tensor_add(out=v_sb[km], in0=ps, in1=bvB)

            # RoPE: a' = a*cos + (Rt.T @ a)*sin on the first 32 partitions
            csl = cos_sb[:, 512 * mb : 512 * (mb + 1)]
            ssl = sin_sb[:, 512 * mb : 512 * (mb + 1)]
            for h in range(HPG):
                for qk in range(2):
                    a = qkT[(2 * h + qk, mb)]
                    ps_r = psum.tile([ROT, 512], F32, tag="score", bufs=3, name="ps_r")
                    nc.tensor.matmul(ps_r, rt_sb, a[0:ROT, :], start=True, stop=True)
                    tq = work.tile([ROT, 512], BF16, tag=f"ropeq{qk}", bufs=1)
                    nc.gpsimd.tensor_mul(out=tq, in0=a[0:ROT, :], in1=csl)
                    ts = work.tile([ROT, 512], BF16, tag=f"ropes{qk}", bufs=1)
                    nc.vector.tensor_mul(out=ts, in0=ps_r, in1=ssl)
                    nc.gpsimd.tensor_add(out=a[0:ROT, :], in0=tq, in1=ts)

        def phase_b(j):
            """attention for q-block j (all heads) + bounce DMA + AllGather"""
            nkm = 4 * j + 4
            for h in range(HPG):
                qt = qkT[(2 * h, j)]

                def mk_u(i):
                    # diagonal blocks only need q-columns >= 128*(i-4j): compute
                    # the triangular remainder, mask only the first 128 columns
                    qoff = max(0, 128 * (i - 4 * j))
                    width = 512 - qoff
                    kt = qkT[(2 * h + 1, i // 4)]
                    ps_s = psum.tile([128, 512], F32, tag="score", bufs=3, name="ps_s")
                    nc.tensor.matmul(
                        ps_s[:, 0:width],
                        kt[:, 128 * (i % 4) : 128 * (i % 4 + 1)],
                        qt[:, qoff:512],
                        start=True,
                        stop=True,
                    )
                    u = work.tile([128, 512], BF16, tag="u", bufs=5, name="u")
                    nc.scalar.activation(
                        out=u[:, 0:width], in_=ps_s[:, 0:width],
                        func=mybir.ActivationFunctionType.Exp, scale=SCALE,
                    )
                    if i >= 4 * j:
                        nc.vector.tensor_mul(
                            out=u[:, 0:128], in0=u[:, 0:128],
                            in1=masks_sb[:, 0, 0:128],
                        )
                    return u, qoff, width

                ps_av = psum.tile([128, 512], F32, tag="av", bufs=2, name="ps_av")
                ps_sum = psum.tile([128, 512], F32, tag="sum", bufs=1, name="ps_sum")
                pipe = [mk_u(0)]
                if nkm > 1:
                    pipe.append(mk_u(1))
                for i in range(nkm):
                    u, qoff, width = pipe.pop(0)
                    if i + 2 < nkm:
                        pipe.append(mk_u(i + 2))
                    nc.tensor.matmul(
                        ps_av[:, qoff:512],
                        v_sb[i][:, 128 * h : 128 * (h + 1)],
                        u[:, 0:width],
                        start=(i == 0),
                        stop=(i == nkm - 1),
                    )
                    nc.tensor.matmul(
                        ps_sum[:, qoff:512],
                        ones_sb,
                        u[:, 0:width],
                        start=(i == 0),
                        stop=(i == nkm - 1),
                    )
                # free the sum bank fast (533ns copy), then slow reciprocal
                sums = work.tile([128, 512], F32, tag="sums", bufs=2, name="sums")
                nc.vector.tensor_copy(out=sums, in_=ps_sum)
                recipB = work.tile([128, 512], F32, tag="recipB", bufs=2, name="recipB")
                nc.vector.reciprocal(out=recipB, in_=sums)
                attn_t = work.tile([128, 512], BF16, tag="attnT", bufs=6, name="attn_t")
                nc.vector.tensor_mul(out=attn_t, in0=ps_av, in1=recipB)
                nc.sync.dma_start(
                    out=bounce[(j, h // 2)][128 * (h % 2) : 128 * (h % 2 + 1), :],
                    in_=attn_t,
                )
                if h % 2 == 1:
                    nc.gpsimd.collective_compute(
                        "AllGather",
                        mybir.AluOpType.bypass,
                        replica_groups=[[0, 1, 2, 3], [4, 5, 6, 7]],
                        ins=[bounce[(j, h // 2)].opt()],
                        outs=[gath[(j, h // 2)].opt()],
                    )

        def phase_c(j):
            """dense for q-block j"""
            # gath half layout: rank r, local head l in {0,1}, gives row block
            # i = 2r + l  <->  hidden block 4r + 2*half + l
            # kb-outer so each gathered tile is fully consumed on arrival:
            # ga needs only 4 bufs, and the 4 output psum banks borrow from
            # the acc + score rings (phase B of this q-block is long done)
            ps_d = [
                psum.tile(
                    [128, 512], F32, tag=("acc" if ob < 2 else "score"),
                    bufs=(2 if ob < 2 else 3), name=f"ps_d{ob}",
                )
                for ob in range(4)
            ]
            idx = 0
            for half in range(2):
                for i in range(8):
                    hd = 4 * (i // 2) + 2 * half + (i % 2)
                    g_t = work.tile([128, 512], BF16, tag="ga", bufs=4, name="ga")
                    nc.sync.dma_start(
                        out=g_t, in_=gath[(j, half)][128 * i : 128 * (i + 1), :]
                    )
                    for ob in range(4):
                        nc.tensor.matmul(
                            ps_d[ob],
                            wd_sb[hd][:, 128 * ob : 128 * (ob + 1)],
                            g_t,
                            start=(idx == 0),
                            stop=(idx == KB - 1),
                        )
                    idx += 1
            for ob in range(4):
                o_sb = work.tile([128, 512], F32, tag="o_sb", bufs=3, name="o_sb")
                nc.vector.tensor_scalar_add(
                    out=o_sb, in0=ps_d[ob], scalar1=bd_sb[:, ob : ob + 1]
                )
                nc.sync.dma_start(
                    out=io["outT"][128 * ob : 128 * (ob + 1), 512 * j : 512 * (j + 1)],
                    in_=o_sb,
                )

        # ============ emission order ============
        # interleave so every consumer is emitted >=1 full phase after its
        # producer: PE never head-of-line blocks on ACT/DVE/collective.
        phase_a(0)
        phase_a(1)
        phase_b(0)
        load_wd()
        phase_a(2)
        phase_b(1)
        phase_a(3)
        phase_b(2)
        phase_c(0)
        phase_c(1)
        phase_b(3)
        phase_c(2)
        phase_c(3)


def _prep_inputs(x, position_ids, Wqkv, bqkv, Wdense, bdense):
    """Host-side sharding + bf16 pre-cast + weight re-layout."""
    bf16 = ml_dtypes.bfloat16
    inv_freq = 1.0 / (BASE ** (np.arange(0, ROT, 2, dtype=np.float32) / ROT))

    # diagonal-block masks: mask[p][kk, qq] = 1 if qq >= kk + 128*p
    kk = np.arange(128)[:, None]
    qq = np.arange(512)[None, :]
    masks = np.stack(
        [(qq >= kk + 128 * p) for p in range(4)], axis=1
    ).astype(bf16)  # [128, 4, 512]

    R = np.zeros((ROT, ROT), np.float32)
    R[np.arange(16), np.arange(16) + 16] = -1.0
    R[np.arange(16) + 16, np.arange(16)] = 1.0
    rt = np.ascontiguousarray(R.T).astype(bf16)

    in_maps = []
    for c in range(NCORES):
        b, g = divmod(c, G)
        heads = range(HPG * g, HPG * (g + 1))
        xTb = np.ascontiguousarray(x[b].T).astype(bf16)  # [HID, S]
        wqk = np.concatenate(
            [Wqkv[:, 384 * h : 384 * h + 256] for h in heads], axis=1
        )  # [HID, 1024]
        # -> [nb, p, kb*128+n]
        wqk = np.ascontiguousarray(
            wqk.reshape(KB, 128, NBQK, 128).transpose(2, 1, 0, 3).reshape(
                NBQK, 128, KB * 128
            )
        ).astype(bf16)
        wv = np.concatenate(
            [Wqkv[:, 384 * h + 256 : 384 * h + 384] for h in heads], axis=1
        ).reshape(KB, 128, 512).astype(bf16)
        bqk = np.concatenate(
            [bqkv[384 * h : 384 * h + 256] for h in heads]
        ).astype(np.float32)
        bv = np.concatenate(
            [bqkv[384 * h + 256 : 384 * h + 384] for h in heads]
        ).astype(np.float32)
        wd = np.ascontiguousarray(Wdense[:, 512 * g : 512 * (g + 1)]).reshape(
            KB, 128, 512
        ).astype(bf16)
        bd = np.ascontiguousarray(bdense[512 * g : 512 * (g + 1)]).astype(np.float32)
        ang = np.outer(inv_freq, position_ids[b].astype(np.float32))  # [16, S]
        cosE = np.concatenate([np.cos(ang)] * 2, axis=0)  # [32, S]
        sinE = np.concatenate([np.sin(ang)] * 2, axis=0)
        in_maps.append(
            {
                "xT": xTb,
                "wqk": wqk,
                "wv": wv,
                "bqk": bqk,
                "bv": bv,
                "wd": wd,
                "bd": bd,
                "cosb": cosE.astype(bf16),
                "sinb": sinE.astype(bf16),
                "rt": rt,
                "masks": masks,
            }
        )
    return in_maps


def _run(in_maps, trace=False):
    if "nc" not in _CACHE:
        _CACHE["nc"] = _build_nc()
    nc = _CACHE["nc"]
    res = bass_utils.run_bass_kernel_spmd(
        nc, in_maps, core_ids=list(range(NCORES)), trace=trace
    )
    return res


def kernel(x, position_ids, attention_mask, Wqkv, bqkv, Wdense, bdense,
           _trace=False, _return_results=False):
    x = np.asarray(x, dtype=np.float32)
    position_ids = np.asarray(position_ids)
    Wqkv = np.asarray(Wqkv, dtype=np.float32)
    bqkv = np.asarray(bqkv, dtype=np.float32)
    Wdense = np.asarray(Wdense, dtype=np.float32)
    bdense = np.asarray(bdense, dtype=np.float32)

    in_maps = _prep_inputs(x, position_ids, Wqkv, bqkv, Wdense, bdense)
    res = _run(in_maps, trace=_trace)

    y = np.empty((B, S, HID), dtype=np.float32)
    for c in range(NCORES):
        b, g = divmod(c, G)
        y[b, :, 512 * g : 512 * (g + 1)] = res.results[c]["outT"].T
    if _return_results:
        return y, res
    return y


# revision 29
# speedup vs baseline: 1.1427x; 1.0163x over previous
"""Causal self-attention (dense transformer block) on 8 Trainium2 NeuronCores.

Problem (hardcoded): B=2, S=2048, HID=2048, NH=16, HS=128, ROT=32 (partial
rotary), causal additive mask, f32 I/O.

Sharding: core c = b*4 + g handles batch b and head-group g (4 heads).
 - Phase A (projections): qkT = (x @ Wqk_shard)^T computed from host-provided
   xT (x[b] transposed, pre-cast bf16) so the contraction dim lands on
   partitions.  Q,K are produced transposed ([d, s]); V natural ([s, d]).
 - RoPE in-place on the first 32 dims of each head of Q,K; rotate_half is a
   32x32 sign-permutation matmul (engine partition ops must be 32-aligned).
 - Phase B (attention): S^T tiles [128 k, 512 q] on PE, exp on ACT (no max
   subtraction; scores are O(5)), causality structural (upper blocks skipped,
   diagonal blocks masked after exp), AV and row-sum accumulated on PE
   (row-sum via all-ones 128x128 stationary => result replicated across
   partitions, no broadcast needed).
 - Per-q-block AllGather (groups of 4 = same batch) overlapped with later
   compute phases; dense (column-parallel Wdense) per q-block right after.
 - Host reassembles the 8 [512, 2048] f32 transposed output slices.
"""

import numpy as np
import ml_dtypes

import concourse.bass as bass
import concourse.bacc as bacc
import concourse.mybir as mybir
import concourse.tile as tile
from concourse import bass_utils

B, S, HID = 2, 2048, 2048
NH = 16
HS = 128
ROT = 32
BASE = 10000.0
G = 4            # head-groups (4 heads each)
HPG = NH // G    # heads per core = 4
NCORES = 8

MB = 4           # m (seq) blocks of 512
KB = 16          # contraction blocks of 128
NBQK = 2 * HPG   # qk feature blocks of 128 (q,k interleaved per head)
F32 = mybir.dt.float32
BF16 = mybir.dt.bfloat16
SCALE = 1.0 / float(np.sqrt(np.float32(HS)))

_CACHE = {}


def _build_nc():
    nc = bacc.Bacc(
        "TRN2", target_bir_lowering=False, debug=False, num_devices=NCORES
    )

    io = {
        "xT": nc.dram_tensor("xT", [HID, S], BF16, kind="ExternalInput"),
        # wqk host-layout: [nb, p, kb*128+n] so each nb is one contiguous DMA
        "wqk": nc.dram_tensor("wqk", [NBQK, 128, KB * 128], BF16, kind="ExternalInput"),
        "wv": nc.dram_tensor("wv", [KB, 128, 512], BF16, kind="ExternalInput"),
        "wd": nc.dram_tensor("wd", [KB, 128, 512], BF16, kind="ExternalInput"),
        "bqk": nc.dram_tensor("bqk", [NBQK * 128], F32, kind="ExternalInput"),
        "bv": nc.dram_tensor("bv", [HPG * 128], F32, kind="ExternalInput"),
        "bd": nc.dram_tensor("bd", [512], F32, kind="ExternalInput"),
        "cosb": nc.dram_tensor("cosb", [ROT, S], BF16, kind="ExternalInput"),
        "sinb": nc.dram_tensor("sinb", [ROT, S], BF16, kind="ExternalInput"),
        "rt": nc.dram_tensor("rt", [ROT, ROT], BF16, kind="ExternalInput"),
        "masks": nc.dram_tensor("masks", [128, 4, 512], BF16, kind="ExternalInput"),
        "outT": nc.dram_tensor("outT", [512, S], F32, kind="ExternalOutput"),
    }

    with tile.TileContext(nc) as tc:
        _kernel_body(tc, io)

    nc.compile()
    return nc


def _kernel_body(tc, io):
    nc = tc.nc
    from contextlib import ExitStack

    with ExitStack() as ctx:
        singles = ctx.enter_context(tc.tile_pool(name="singles", bufs=1))
        wpool = ctx.enter_context(tc.tile_pool(name="wpool", bufs=1))
        work = ctx.enter_context(tc.tile_pool(name="work", bufs=2))
        psum = ctx.enter_context(tc.tile_pool(name="psum", bufs=2, space="PSUM"))
        dram = ctx.enter_context(tc.tile_pool(name="dram", bufs=1, space="DRAM"))

        # ---- weights for the very first matmuls, in load order ----
        # (first qk weight block, then the first m-block of x, then the rest:
        # the first PE matmul needs only wqk[0] + xbf(0))
        wqk_sb = []
        for nb in range(NBQK):
            w_t = wpool.tile([128, KB, 128], BF16, tag=f"wqk{nb}", name=f"wqk{nb}")
            nc.sync.dma_start(out=w_t, in_=io["wqk"][nb])
            wqk_sb.append(w_t)
            if nb == 0:
                xbf0 = []
                for kb in range(KB):
                    xb = work.tile([128, 512], BF16, tag="xbf", bufs=32, name="xb")
                    nc.sync.dma_start(
                        out=xb, in_=io["xT"][128 * kb : 128 * (kb + 1), 0:512]
                    )
                    xbf0.append(xb)

        # ---- constants ----
        cos_sb = singles.tile([ROT, S], BF16, tag="cos_sb")
        nc.sync.dma_start(out=cos_sb, in_=io["cosb"][:, :])
        sin_sb = singles.tile([ROT, S], BF16, tag="sin_sb")
        nc.sync.dma_start(out=sin_sb, in_=io["sinb"][:, :])
        rt_sb = singles.tile([ROT, ROT], BF16, tag="rt_sb")
        nc.sync.dma_start(out=rt_sb, in_=io["rt"][:, :])
        masks_sb = singles.tile([128, 4, 512], BF16, tag="masks_sb")
        nc.sync.dma_start(out=masks_sb, in_=io["masks"][:, :, :])
        ones_sb = singles.tile([128, 128], BF16, tag="ones_sb")
        nc.vector.memset(ones_sb, 1.0)
        bqk_sb = singles.tile([128, NBQK], F32, tag="bqk_sb")
        nc.sync.dma_start(out=bqk_sb, in_=io["bqk"].rearrange("(n p) -> p n", p=128))
        bd_sb = singles.tile([128, 4], F32, tag="bd_sb")
        nc.sync.dma_start(out=bd_sb, in_=io["bd"].rearrange("(n p) -> p n", p=128))
        bv_row = singles.tile([1, HPG * 128], F32, tag="bv_row")
        nc.sync.dma_start(out=bv_row, in_=io["bv"][None, :])
        bvB = singles.tile([128, HPG * 128], F32, tag="bvB")
        nc.gpsimd.partition_broadcast(bvB, bv_row)

        wv_sb = []
        wd_sb = []

        def load_wv():
            for kb in range(KB):
                wv_t = wpool.tile([128, 512], BF16, tag=f"wv{kb}", name=f"wv{kb}")
                nc.sync.dma_start(out=wv_t, in_=io["wv"][kb])
                wv_sb.append(wv_t)

        def load_wd():
            for kb in range(KB):
                wd_t = wpool.tile([128, 512], BF16, tag=f"wd{kb}", name=f"wd{kb}")
                nc.sync.dma_start(out=wd_t, in_=io["wd"][kb])
                wd_sb.append(wd_t)

        # ---- DRAM bounce buffers, one per (q-block, head-pair) ----
        # splitting the AllGather in half lets it start as soon as the first
        # two heads of a q-block are done, and halves the tail of the last one
        bounce = {}
        gath = {}
        for j in range(MB):
            for half in range(2):
                bounce[(j, half)] = dram.tile(
                    [256, 512], BF16, tag=f"bounce{j}_{half}", name=f"bounce{j}_{half}"
                )
                gath[(j, half)] = dram.tile(
                    [1024, 512], BF16, tag=f"gath{j}_{half}", name=f"gath{j}_{half}"
                )

        # ---- persistent qkT / V tiles ----
        qkT = {}
        for nb in range(NBQK):
            for mb in range(MB):
                qkT[(nb, mb)] = wpool.tile(
                    [128, 512], BF16, tag=f"qkT_{nb}_{mb}", name=f"qkT_{nb}_{mb}"
                )
        v_sb = []
        for km in range(KB):
            v_sb.append(
                wpool.tile([128, HPG * 128], BF16, tag=f"v_{km}", name=f"v_{km}")
            )

        # ============ phase emitters ============

        def phase_a(mb):
            """projections for m-block mb + RoPE"""
            if mb == 0:
                xbf = xbf0
                load_wv()
            else:
                xbf = []
                for kb in range(KB):
                    xb = work.tile([128, 512], BF16, tag="xbf", bufs=32, name="xb")
                    nc.sync.dma_start(
                        out=xb,
                        in_=io["xT"][
                            128 * kb : 128 * (kb + 1), 512 * mb : 512 * (mb + 1)
                        ],
                    )
                    xbf.append(xb)

            for nb in range(NBQK):
                ps = psum.tile([128, 512], F32, tag="acc", bufs=2, name="ps_qk")
                for kb in range(KB):
                    nc.tensor.matmul(
                        ps,
                        wqk_sb[nb][:, kb, :],
                        xbf[kb],
                        start=(kb == 0),
                        stop=(kb == KB - 1),
                    )
                nc.vector.tensor_scalar_add(
                    out=qkT[(nb, mb)], in0=ps, scalar1=bqk_sb[:, nb : nb + 1]
                )

            for msub in range(4):
                km = 4 * mb + msub
                ps = psum.tile([128, HPG * 128], F32, tag="acc", bufs=2, name="ps_v")
                for kb in range(KB):
                    nc.tensor.matmul(
                        ps,
                        xbf[kb][:, 128 * msub : 128 * (msub + 1)],
                        wv_sb[kb],
                        start=(kb == 0),
                        stop=(kb == KB - 1),
                    )
                nc.vector.tensor_add(out=v_sb[km], in0=ps, in1=bvB)

            # RoPE: a' = a*cos + (Rt.T @ a)*sin on the first 32 partitions
            csl = cos_sb[:, 512 * mb : 512 * (mb + 1)]
            ssl = sin_sb[:, 512 * mb : 512 * (mb + 1)]
            for h in range(HPG):
                for qk in range(2):
                    a = qkT[(2 * h + qk, mb)]
                    ps_r = psum.tile([ROT, 512], F32, tag="score", bufs=3, name="ps_r")
                    nc.tensor.matmul(ps_r, rt_sb, a[0:ROT, :], start=True, stop=True)
                    tq = work.tile([ROT, 512], BF16, tag=f"ropeq{qk}", bufs=1)
                    nc.gpsimd.tensor_mul(out=tq, in0=a[0:ROT, :], in1=csl)
                    ts = work.tile([ROT, 512], BF16, tag=f"ropes{qk}", bufs=1)
                    nc.vector.tensor_mul(out=ts, in0=ps_r, in1=ssl)
                    nc.gpsimd.tensor_add(out=a[0:ROT, :], in0=tq, in1=ts)

        def phase_b(j):
            """attention for q-block j (all heads) + bounce DMA + AllGather"""
            nkm = 4 * j + 4
            for h in range(HPG):
                qt = qkT[(2 * h, j)]

                def mk_u(i):
                    # diagonal blocks only need q-columns >= 128*(i-4j): compute
                    # the triangular remainder, mask only the first 128 columns
                    qoff = max(0, 128 * (i - 4 * j))
                    width = 512 - qoff
                    kt = qkT[(2 * h + 1, i // 4)]
                    ps_s = psum.tile([128, 512], F32, tag="score", bufs=3, name="ps_s")
                    nc.tensor.matmul(
                        ps_s[:, 0:width],
                        kt[:, 128 * (i % 4) : 128 * (i % 4 + 1)],
                        qt[:, qoff:512],
                        start=True,
                        stop=True,
                    )
                    u = work.tile([128, 512], BF16, tag="u", bufs=5, name="u")
                    nc.scalar.activation(
                        out=u[:, 0:width], in_=ps_s[:, 0:width],
                        func=mybir.ActivationFunctionType.Exp, scale=SCALE,
                    )
                    if i >= 4 * j:
                        nc.vector.tensor_mul(
                            out=u[:, 0:128], in0=u[:, 0:128],
                            in1=masks_sb[:, 0, 0:128],
                        )
                    return u, qoff, width

                ps_av = psum.tile([128, 512], F32, tag="av", bufs=2, name="ps_av")
                ps_sum = psum.tile([128, 512], F32, tag="sum", bufs=1, name="ps_sum")
                pipe = [mk_u(0)]
                if nkm > 1:
                    pipe.append(mk_u(1))
                for i in range(nkm):
                    u, qoff, width = pipe.pop(0)
                    if i + 2 < nkm:
                        pipe.append(mk_u(i + 2))
                    nc.tensor.matmul(
                        ps_av[:, qoff:512],
                        v_sb[i][:, 128 * h : 128 * (h + 1)],
                        u[:, 0:width],
                        start=(i == 0),
                        stop=(i == nkm - 1),
                    )
                    nc.tensor.matmul(
                        ps_sum[:, qoff:512],
                        ones_sb,
                        u[:, 0:width],
                        start=(i == 0),
                        stop=(i == nkm - 1),
                    )
                # ~18-bit reciprocal, 5x faster than reciprocal(): plenty for
                # softmax denominators (well away from 0/inf edge cases), and
                # keeps the DVE queue from head-of-line blocking the PE
                recipB = work.tile([128, 512], F32, tag="recipB", bufs=2, name="recipB")
                nc.vector.reciprocal_approx_fast(out=recipB, in_=ps_sum)
                attn_t = work.tile([128, 512], BF16, tag="attnT", bufs=6, name="attn_t")
                nc.vector.tensor_mul(out=attn_t, in0=ps_av, in1=recipB)
                nc.sync.dma_start(
                    out=bounce[(j, h // 2)][128 * (h % 2) : 128 * (h % 2 + 1), :],
                    in_=attn_t,
                )
                if h % 2 == 1:
                    nc.gpsimd.collective_compute(
                        "AllGather",
                        mybir.AluOpType.bypass,
                        replica_groups=[[0, 1, 2, 3], [4, 5, 6, 7]],
                        ins=[bounce[(j, h // 2)].opt()],
                        outs=[gath[(j, h // 2)].opt()],
                    )

        def phase_c(j):
            """dense for q-block j"""
            # gath half layout: rank r, local head l in {0,1}, gives row block
            # i = 2r + l  <->  hidden block 4r + 2*half + l
            # kb-outer so each gathered tile is fully consumed on arrival:
            # ga needs only 4 bufs, and the 4 output psum banks borrow from
            # the acc + score rings (phase B of this q-block is long done)
            ps_d = [
                psum.tile(
                    [128, 512], F32, tag=("acc" if ob < 2 else "score"),
                    bufs=(2 if ob < 2 else 3), name=f"ps_d{ob}",
                )
                for ob in range(4)
            ]
            idx = 0
            for half in range(2):
                for i in range(8):
                    hd = 4 * (i // 2) + 2 * half + (i % 2)
                    g_t = work.tile([128, 512], BF16, tag="ga", bufs=4, name="ga")
                    nc.sync.dma_start(
                        out=g_t, in_=gath[(j, half)][128 * i : 128 * (i + 1), :]
                    )
                    for ob in range(4):
                        nc.tensor.matmul(
                            ps_d[ob],
                            wd_sb[hd][:, 128 * ob : 128 * (ob + 1)],
                            g_t,
                            start=(idx == 0),
                            stop=(idx == KB - 1),
                        )
                    idx += 1
            for ob in range(4):
                o_sb = work.tile([128, 512], F32, tag="o_sb", bufs=3, name="o_sb")
                nc.vector.tensor_scalar_add(
                    out=o_sb, in0=ps_d[ob], scalar1=bd_sb[:, ob : ob + 1]
                )
                nc.sync.dma_start(
                    out=io["outT"][128 * ob : 128 * (ob + 1), 512 * j : 512 * (j + 1)],
                    in_=o_sb,
                )

        # ============ emission order ============
        # interleave so every consumer is emitted >=1 full phase after its
        # producer: PE never head-of-line blocks on ACT/DVE/collective.
        phase_a(0)
        phase_a(1)
        phase_b(0)
        load_wd()
        phase_a(2)
        phase_b(1)
        phase_a(3)
        phase_b(2)
        phase_c(0)
        phase_c(1)
        phase_b(3)
        phase_c(2)
        phase_c(3)


def _prep_inputs(x, position_ids, Wqkv, bqkv, Wdense, bdense):
    """Host-side sharding + bf16 pre-cast + weight re-layout."""
    bf16 = ml_dtypes.bfloat16
    inv_freq = 1.0 / (BASE ** (np.arange(0, ROT, 2, dtype=np.float32) / ROT))

    # diagonal-block masks: mask[p][kk, qq] = 1 if qq >= kk + 128*p
    kk = np.arange(128)[:, None]
    qq = np.arange(512)[None, :]
    masks = np.stack(
        [(qq >= kk + 128 * p) for p in range(4)], axis=1
    ).astype(bf16)  # [128, 4, 512]

    R = np.zeros((ROT, ROT), np.float32)
    R[np.arange(16), np.arange(16) + 16] = -1.0
    R[np.arange(16) + 16, np.arange(16)] = 1.0
    rt = np.ascontiguousarray(R.T).astype(bf16)

    in_maps = []
    for c in range(NCORES):
        b, g = divmod(c, G)
        heads = range(HPG * g, HPG * (g + 1))
        xTb = np.ascontiguousarray(x[b].T).astype(bf16)  # [HID, S]
        wqk = np.concatenate(
            [Wqkv[:, 384 * h : 384 * h + 256] for h in heads], axis=1
        )  # [HID, 1024]
        # -> [nb, p, kb*128+n]
        wqk = np.ascontiguousarray(
            wqk.reshape(KB, 128, NBQK, 128).transpose(2, 1, 0, 3).reshape(
                NBQK, 128, KB * 128
            )
        ).astype(bf16)
        wv = np.concatenate(
            [Wqkv[:, 384 * h + 256 : 384 * h + 384] for h in heads], axis=1
        ).reshape(KB, 128, 512).astype(bf16)
        bqk = np.concatenate(
            [bqkv[384 * h : 384 * h + 256] for h in heads]
        ).astype(np.float32)
        bv = np.concatenate(
            [bqkv[384 * h + 256 : 384 * h + 384] for h in heads]
        ).astype(np.float32)
        wd = np.ascontiguousarray(Wdense[:, 512 * g : 512 * (g + 1)]).reshape(
            KB, 128, 512
        ).astype(bf16)
        bd = np.ascontiguousarray(bdense[512 * g : 512 * (g + 1)]).astype(np.float32)
        ang = np.outer(inv_freq, position_ids[b].astype(np.float32))  # [16, S]
        cosE = np.concatenate([np.cos(ang)] * 2, axis=0)  # [32, S]
        sinE = np.concatenate([np.sin(ang)] * 2, axis=0)
        in_maps.append(
            {
                "xT": xTb,
                "wqk": wqk,
                "wv": wv,
                "bqk": bqk,
                "bv": bv,
                "wd": wd,
                "bd": bd,
                "cosb": cosE.astype(bf16),
                "sinb": sinE.astype(bf16),
                "rt": rt,
                "masks": masks,
            }
        )
    return in_maps


def _run(in_maps, trace=False):
    if "nc" not in _CACHE:
        _CACHE["nc"] = _build_nc()
    nc = _CACHE["nc"]
    res = bass_utils.run_bass_kernel_spmd(
        nc, in_maps, core_ids=list(range(NCORES)), trace=trace
    )
    return res


def kernel(x, position_ids, attention_mask, Wqkv, bqkv, Wdense, bdense,
           _trace=False, _return_results=False):
    x = np.asarray(x, dtype=np.float32)
    position_ids = np.asarray(position_ids)
    Wqkv = np.asarray(Wqkv, dtype=np.float32)
    bqkv = np.asarray(bqkv, dtype=np.float32)
    Wdense = np.asarray(Wdense, dtype=np.float32)
    bdense = np.asarray(bdense, dtype=np.float32)

    in_maps = _prep_inputs(x, position_ids, Wqkv, bqkv, Wdense, bdense)
    res = _run(in_maps, trace=_trace)

    y = np.empty((B, S, HID), dtype=np.float32)
    for c in range(NCORES):
        b, g = divmod(c, G)
        y[b, :, 512 * g : 512 * (g + 1)] = res.results[c]["outT"].T
    if _return_results:
        return y, res
    return y


# revision 31
# speedup vs baseline: 1.1437x; 1.0008x over previous
"""Causal self-attention (dense transformer block) on 8 Trainium2 NeuronCores.

Problem (hardcoded): B=2, S=2048, HID=2048, NH=16, HS=128, ROT=32 (partial
rotary), causal additive mask, f32 I/O.

Sharding: core c = b*4 + g handles batch b and head-group g (4 heads).
 - Phase A (projections): qkT = (x @ Wqk_shard)^T computed from host-provided
   xT (x[b] transposed, pre-cast bf16) so the contraction dim lands on
   partitions.  Q,K are produced transposed ([d, s]); V natural ([s, d]).
 - RoPE in-place on the first 32 dims of each head of Q,K; rotate_half is a
   32x32 sign-permutation matmul (engine partition ops must be 32-aligned).
 - Phase B (attention): S^T tiles [128 k, 512 q] on PE, exp on ACT (no max
   subtraction; scores are O(5)), causality structural (upper blocks skipped,
   diagonal blocks masked after exp), AV and row-sum accumulated on PE
   (row-sum via all-ones 128x128 stationary => result replicated across
   partitions, no broadcast needed).
 - Per-q-block AllGather (groups of 4 = same batch) overlapped with later
   compute phases; dense (column-parallel Wdense) per q-block right after.
 - Host reassembles the 8 [512, 2048] f32 transposed output slices.
"""

import numpy as np
import ml_dtypes

import concourse.bass as bass
import concourse.bacc as bacc
import concourse.mybir as mybir
import concourse.tile as tile
from concourse import bass_utils

B, S, HID = 2, 2048, 2048
NH = 16
HS = 128
ROT = 32
BASE = 10000.0
G = 4            # head-groups (4 heads each)
HPG = NH // G    # heads per core = 4
NCORES = 8

MB = 4           # m (seq) blocks of 512
KB = 16          # contraction blocks of 128
NBQK = 2 * HPG   # qk feature blocks of 128 (q,k interleaved per head)
F32 = mybir.dt.float32
BF16 = mybir.dt.bfloat16
SCALE = 1.0 / float(np.sqrt(np.float32(HS)))

_CACHE = {}


def _build_nc():
    nc = bacc.Bacc(
        "TRN2", target_bir_lowering=False, debug=False, num_devices=NCORES
    )

    io = {
        "xT": nc.dram_tensor("xT", [HID, S], BF16, kind="ExternalInput"),
        # wqk host-layout: [nb, p, kb*128+n] so each nb is one contiguous DMA
        "wqk": nc.dram_tensor("wqk", [NBQK, 128, KB * 128], BF16, kind="ExternalInput"),
        "wv": nc.dram_tensor("wv", [KB, 128, 512], BF16, kind="ExternalInput"),
        "wd": nc.dram_tensor("wd", [KB, 128, 512], BF16, kind="ExternalInput"),
        "bqk": nc.dram_tensor("bqk", [NBQK * 128], F32, kind="ExternalInput"),
        "bv": nc.dram_tensor("bv", [HPG * 128], F32, kind="ExternalInput"),
        "bd": nc.dram_tensor("bd", [512], F32, kind="ExternalInput"),
        "cosb": nc.dram_tensor("cosb", [ROT, S], BF16, kind="ExternalInput"),
        "sinb": nc.dram_tensor("sinb", [ROT, S], BF16, kind="ExternalInput"),
        "rt": nc.dram_tensor("rt", [ROT, ROT], BF16, kind="ExternalInput"),
        "masks": nc.dram_tensor("masks", [128, 4, 512], BF16, kind="ExternalInput"),
        "outT": nc.dram_tensor("outT", [512, S], F32, kind="ExternalOutput"),
    }

    with tile.TileContext(nc) as tc:
        _kernel_body(tc, io)

    nc.compile()
    return nc


def _kernel_body(tc, io):
    nc = tc.nc
    from contextlib import ExitStack

    with ExitStack() as ctx:
        singles = ctx.enter_context(tc.tile_pool(name="singles", bufs=1))
        wpool = ctx.enter_context(tc.tile_pool(name="wpool", bufs=1))
        work = ctx.enter_context(tc.tile_pool(name="work", bufs=2))
        psum = ctx.enter_context(tc.tile_pool(name="psum", bufs=2, space="PSUM"))
        dram = ctx.enter_context(tc.tile_pool(name="dram", bufs=1, space="DRAM"))

        # ---- tiny constants first: the nb=0 bias-add gates the acc-psum
        # ring, so these must not queue behind megabytes of weight DMAs ----
        bqk_sb = singles.tile([128, NBQK], F32, tag="bqk_sb")
        nc.sync.dma_start(out=bqk_sb, in_=io["bqk"].rearrange("(n p) -> p n", p=128))
        bd_sb = singles.tile([128, 4], F32, tag="bd_sb")
        nc.sync.dma_start(out=bd_sb, in_=io["bd"].rearrange("(n p) -> p n", p=128))
        bv_row = singles.tile([1, HPG * 128], F32, tag="bv_row")
        nc.sync.dma_start(out=bv_row, in_=io["bv"][None, :])
        bvB = singles.tile([128, HPG * 128], F32, tag="bvB")
        nc.gpsimd.partition_broadcast(bvB, bv_row)
        rt_sb = singles.tile([ROT, ROT], BF16, tag="rt_sb")
        nc.sync.dma_start(out=rt_sb, in_=io["rt"][:, :])
        cos_sb = singles.tile([ROT, S], BF16, tag="cos_sb")
        nc.sync.dma_start(out=cos_sb, in_=io["cosb"][:, :])
        sin_sb = singles.tile([ROT, S], BF16, tag="sin_sb")
        nc.sync.dma_start(out=sin_sb, in_=io["sinb"][:, :])
        masks_sb = singles.tile([128, 4, 512], BF16, tag="masks_sb")
        nc.sync.dma_start(out=masks_sb, in_=io["masks"][:, :, :])
        ones_sb = singles.tile([128, 128], BF16, tag="ones_sb")
        nc.vector.memset(ones_sb, 1.0)

        # ---- weights for the very first matmuls, in load order ----
        # (first qk weight block, then the first m-block of x, then the rest:
        # the first PE matmul needs only wqk[0] + xbf(0))
        wqk_sb = []
        for nb in range(NBQK):
            w_t = wpool.tile([128, KB, 128], BF16, tag=f"wqk{nb}", name=f"wqk{nb}")
            nc.sync.dma_start(out=w_t, in_=io["wqk"][nb])
            wqk_sb.append(w_t)
            if nb == 0:
                xbf0 = []
                for kb in range(KB):
                    xb = work.tile([128, 512], BF16, tag="xbf", bufs=32, name="xb")
                    nc.sync.dma_start(
                        out=xb, in_=io["xT"][128 * kb : 128 * (kb + 1), 0:512]
                    )
                    xbf0.append(xb)

        wv_sb = []
        wd_sb = []

        def load_wv():
            for kb in range(KB):
                wv_t = wpool.tile([128, 512], BF16, tag=f"wv{kb}", name=f"wv{kb}")
                nc.sync.dma_start(out=wv_t, in_=io["wv"][kb])
                wv_sb.append(wv_t)

        def load_wd():
            for kb in range(KB):
                wd_t = wpool.tile([128, 512], BF16, tag=f"wd{kb}", name=f"wd{kb}")
                nc.sync.dma_start(out=wd_t, in_=io["wd"][kb])
                wd_sb.append(wd_t)

        # ---- DRAM bounce buffers, one per (q-block, head-pair) ----
        # splitting the AllGather in half lets it start as soon as the first
        # two heads of a q-block are done, and halves the tail of the last one
        bounce = {}
        gath = {}
        for j in range(MB):
            for half in range(2):
                bounce[(j, half)] = dram.tile(
                    [256, 512], BF16, tag=f"bounce{j}_{half}", name=f"bounce{j}_{half}"
                )
                gath[(j, half)] = dram.tile(
                    [1024, 512], BF16, tag=f"gath{j}_{half}", name=f"gath{j}_{half}"
                )

        # ---- persistent qkT / V tiles ----
        qkT = {}
        for nb in range(NBQK):
            for mb in range(MB):
                qkT[(nb, mb)] = wpool.tile(
                    [128, 512], BF16, tag=f"qkT_{nb}_{mb}", name=f"qkT_{nb}_{mb}"
                )
        v_sb = []
        for km in range(KB):
            v_sb.append(
                wpool.tile([128, HPG * 128], BF16, tag=f"v_{km}", name=f"v_{km}")
            )

        # ============ phase emitters ============

        def phase_a(mb):
            """projections for m-block mb + RoPE"""
            if mb == 0:
                xbf = xbf0
                load_wv()
            else:
                xbf = []
                for kb in range(KB):
                    xb = work.tile([128, 512], BF16, tag="xbf", bufs=32, name="xb")
                    nc.sync.dma_start(
                        out=xb,
                        in_=io["xT"][
                            128 * kb : 128 * (kb + 1), 512 * mb : 512 * (mb + 1)
                        ],
                    )
                    xbf.append(xb)

            for nb in range(NBQK):
                ps = psum.tile([128, 512], F32, tag="acc", bufs=2, name="ps_qk")
                for kb in range(KB):
                    nc.tensor.matmul(
                        ps,
                        wqk_sb[nb][:, kb, :],
                        xbf[kb],
                        start=(kb == 0),
                        stop=(kb == KB - 1),
                    )
                nc.vector.tensor_scalar_add(
                    out=qkT[(nb, mb)], in0=ps, scalar1=bqk_sb[:, nb : nb + 1]
                )

            for msub in range(4):
                km = 4 * mb + msub
                ps = psum.tile([128, HPG * 128], F32, tag="acc", bufs=2, name="ps_v")
                for kb in range(KB):
                    nc.tensor.matmul(
                        ps,
                        xbf[kb][:, 128 * msub : 128 * (msub + 1)],
                        wv_sb[kb],
                        start=(kb == 0),
                        stop=(kb == KB - 1),
                    )
                nc.vector.tensor_add(out=v_sb[km], in0=ps, in1=bvB)

            # RoPE: a' = a*cos + (Rt.T @ a)*sin on the first 32 partitions
            csl = cos_sb[:, 512 * mb : 512 * (mb + 1)]
            ssl = sin_sb[:, 512 * mb : 512 * (mb + 1)]
            for h in range(HPG):
                for qk in range(2):
                    a = qkT[(2 * h + qk, mb)]
                    ps_r = psum.tile([ROT, 512], F32, tag="score", bufs=3, name="ps_r")
                    nc.tensor.matmul(ps_r, rt_sb, a[0:ROT, :], start=True, stop=True)
                    tq = work.tile([ROT, 512], BF16, tag=f"ropeq{qk}", bufs=1)
                    nc.gpsimd.tensor_mul(out=tq, in0=a[0:ROT, :], in1=csl)
                    ts = work.tile([ROT, 512], BF16, tag=f"ropes{qk}", bufs=1)
                    nc.vector.tensor_mul(out=ts, in0=ps_r, in1=ssl)
                    nc.gpsimd.tensor_add(out=a[0:ROT, :], in0=tq, in1=ts)

        def phase_b(j):
            """attention for q-block j (all heads) + bounce DMA + AllGather"""
            nkm = 4 * j + 4
            for h in range(HPG):
                qt = qkT[(2 * h, j)]

                def mk_u(i):
                    # diagonal blocks only need q-columns >= 128*(i-4j): compute
                    # the triangular remainder, mask only the first 128 columns
                    qoff = max(0, 128 * (i - 4 * j))
                    width = 512 - qoff
                    kt = qkT[(2 * h + 1, i // 4)]
                    ps_s = psum.tile([128, 512], F32, tag="score", bufs=3, name="ps_s")
                    nc.tensor.matmul(
                        ps_s[:, 0:width],
                        kt[:, 128 * (i % 4) : 128 * (i % 4 + 1)],
                        qt[:, qoff:512],
                        start=True,
                        stop=True,
                    )
                    u = work.tile([128, 512], BF16, tag="u", bufs=5, name="u")
                    nc.scalar.activation(
                        out=u[:, 0:width], in_=ps_s[:, 0:width],
                        func=mybir.ActivationFunctionType.Exp, scale=SCALE,
                    )
                    if i >= 4 * j:
                        nc.vector.tensor_mul(
                            out=u[:, 0:128], in0=u[:, 0:128],
                            in1=masks_sb[:, 0, 0:128],
                        )
                    return u, qoff, width

                ps_av = psum.tile([128, 512], F32, tag="av", bufs=2, name="ps_av")
                ps_sum = psum.tile([128, 512], F32, tag="sum", bufs=1, name="ps_sum")
                pipe = [mk_u(0)]
                if nkm > 1:
                    pipe.append(mk_u(1))
                for i in range(nkm):
                    u, qoff, width = pipe.pop(0)
                    if i + 2 < nkm:
                        pipe.append(mk_u(i + 2))
                    nc.tensor.matmul(
                        ps_av[:, qoff:512],
                        v_sb[i][:, 128 * h : 128 * (h + 1)],
                        u[:, 0:width],
                        start=(i == 0),
                        stop=(i == nkm - 1),
                    )
                    nc.tensor.matmul(
                        ps_sum[:, qoff:512],
                        ones_sb,
                        u[:, 0:width],
                        start=(i == 0),
                        stop=(i == nkm - 1),
                    )
                # ~18-bit reciprocal, 5x faster than reciprocal(): plenty for
                # softmax denominators (well away from 0/inf edge cases), and
                # keeps the DVE queue from head-of-line blocking the PE
                recipB = work.tile([128, 512], F32, tag="recipB", bufs=2, name="recipB")
                nc.vector.reciprocal_approx_fast(out=recipB, in_=ps_sum)
                attn_t = work.tile([128, 512], BF16, tag="attnT", bufs=6, name="attn_t")
                nc.vector.tensor_mul(out=attn_t, in0=ps_av, in1=recipB)
                nc.sync.dma_start(
                    out=bounce[(j, h // 2)][128 * (h % 2) : 128 * (h % 2 + 1), :],
                    in_=attn_t,
                )
                if h % 2 == 1:
                    nc.gpsimd.collective_compute(
                        "AllGather",
                        mybir.AluOpType.bypass,
                        replica_groups=[[0, 1, 2, 3], [4, 5, 6, 7]],
                        ins=[bounce[(j, h // 2)].opt()],
                        outs=[gath[(j, h // 2)].opt()],
                    )

        def phase_c(j):
            """dense for q-block j"""
            # gath half layout: rank r, local head l in {0,1}, gives row block
            # i = 2r + l  <->  hidden block 4r + 2*half + l
            # kb-outer so each gathered tile is fully consumed on arrival:
            # ga needs only 4 bufs, and the 4 output psum banks borrow from
            # the acc + score rings (phase B of this q-block is long done)
            ps_d = [
                psum.tile(
                    [128, 512], F32, tag=("acc" if ob < 2 else "score"),
                    bufs=(2 if ob < 2 else 3), name=f"ps_d{ob}",
                )
                for ob in range(4)
            ]
            idx = 0
            for half in range(2):
                for i in range(8):
                    hd = 4 * (i // 2) + 2 * half + (i % 2)
                    g_t = work.tile([128, 512], BF16, tag="ga", bufs=4, name="ga")
                    nc.sync.dma_start(
                        out=g_t, in_=gath[(j, half)][128 * i : 128 * (i + 1), :]
                    )
                    for ob in range(4):
                        nc.tensor.matmul(
                            ps_d[ob],
                            wd_sb[hd][:, 128 * ob : 128 * (ob + 1)],
                            g_t,
                            start=(idx == 0),
                            stop=(idx == KB - 1),
                        )
                    idx += 1
            for ob in range(4):
                o_sb = work.tile([128, 512], F32, tag="o_sb", bufs=3, name="o_sb")
                nc.vector.tensor_scalar_add(
                    out=o_sb, in0=ps_d[ob], scalar1=bd_sb[:, ob : ob + 1]
                )
                nc.sync.dma_start(
                    out=io["outT"][128 * ob : 128 * (ob + 1), 512 * j : 512 * (j + 1)],
                    in_=o_sb,
                )

        # ============ emission order ============
        # interleave so every consumer is emitted >=1 full phase after its
        # producer: PE never head-of-line blocks on ACT/DVE/collective.
        phase_a(0)
        phase_a(1)
        phase_b(0)
        load_wd()
        phase_a(2)
        phase_b(1)
        phase_a(3)
        phase_b(2)
        phase_b(3)
        phase_c(0)
        phase_c(1)
        phase_c(2)
        phase_c(3)


def _prep_inputs(x, position_ids, Wqkv, bqkv, Wdense, bdense):
    """Host-side sharding + bf16 pre-cast + weight re-layout."""
    bf16 = ml_dtypes.bfloat16
    inv_freq = 1.0 / (BASE ** (np.arange(0, ROT, 2, dtype=np.float32) / ROT))

    # diagonal-block masks: mask[p][kk, qq] = 1 if qq >= kk + 128*p
    kk = np.arange(128)[:, None]
    qq = np.arange(512)[None, :]
    masks = np.stack(
        [(qq >= kk + 128 * p) for p in range(4)], axis=1
    ).astype(bf16)  # [128, 4, 512]

    R = np.zeros((ROT, ROT), np.float32)
    R[np.arange(16), np.arange(16) + 16] = -1.0
    R[np.arange(16) + 16, np.arange(16)] = 1.0
    rt = np.ascontiguousarray(R.T).astype(bf16)

    in_maps = []
    for c in range(NCORES):
        b, g = divmod(c, G)
        heads = range(HPG * g, HPG * (g + 1))
        xTb = np.ascontiguousarray(x[b].T).astype(bf16)  # [HID, S]
        wqk = np.concatenate(
            [Wqkv[:, 384 * h : 384 * h + 256] for h in heads], axis=1
        )  # [HID, 1024]
        # -> [nb, p, kb*128+n]
        wqk = np.ascontiguousarray(
            wqk.reshape(KB, 128, NBQK, 128).transpose(2, 1, 0, 3).reshape(
                NBQK, 128, KB * 128
            )
        ).astype(bf16)
        wv = np.concatenate(
            [Wqkv[:, 384 * h + 256 : 384 * h + 384] for h in heads], axis=1
        ).reshape(KB, 128, 512).astype(bf16)
        bqk = np.concatenate(
            [bqkv[384 * h : 384 * h + 256] for h in heads]
        ).astype(np.float32)
        bv = np.concatenate(
            [bqkv[384 * h + 256 : 384 * h + 384] for h in heads]
        ).astype(np.float32)
        wd = np.ascontiguousarray(Wdense[:, 512 * g : 512 * (g + 1)]).reshape(
            KB, 128, 512
        ).astype(bf16)
        bd = np.ascontiguousarray(bdense[512 * g : 512 * (g + 1)]).astype(np.float32)
        ang = np.outer(inv_freq, position_ids[b].astype(np.float32))  # [16, S]
        cosE = np.concatenate([np.cos(ang)] * 2, axis=0)  # [32, S]
        sinE = np.concatenate([np.sin(ang)] * 2, axis=0)
        in_maps.append(
            {
                "xT": xTb,
                "wqk": wqk,
                "wv": wv,
                "bqk": bqk,
                "bv": bv,
                "wd": wd,
                "bd": bd,
                "cosb": cosE.astype(bf16),
                "sinb": sinE.astype(bf16),
                "rt": rt,
                "masks": masks,
            }
        )
    return in_maps


def _run(in_maps, trace=False):
    if "nc" not in _CACHE:
        _CACHE["nc"] = _build_nc()
    nc = _CACHE["nc"]
    res = bass_utils.run_bass_kernel_spmd(
        nc, in_maps, core_ids=list(range(NCORES)), trace=trace
    )
    return res


def kernel(x, position_ids, attention_mask, Wqkv, bqkv, Wdense, bdense,
           _trace=False, _return_results=False):
    x = np.asarray(x, dtype=np.float32)
    position_ids = np.asarray(position_ids)
    Wqkv = np.asarray(Wqkv, dtype=np.float32)
    bqkv = np.asarray(bqkv, dtype=np.float32)
    Wdense = np.asarray(Wdense, dtype=np.float32)
    bdense = np.asarray(bdense, dtype=np.float32)

    in_maps = _prep_inputs(x, position_ids, Wqkv, bqkv, Wdense, bdense)
    res = _run(in_maps, trace=_trace)

    y = np.empty((B, S, HID), dtype=np.float32)
    for c in range(NCORES):
        b, g = divmod(c, G)
        y[b, :, 512 * g : 512 * (g + 1)] = res.results[c]["outT"].T
    if _return_results:
        return y, res
    return y
